# revision 22
# baseline (speedup 1.0000x reference)
"""Trainium2 Bass kernel for nn_Attention (dense multi-head cross-attention).

Problem: B=8 batches, N=M=2048 seq, D=512 hidden, H=8 heads.
  per head h: k_h = k @ Wk[h] + bk[h]; v_h, q_h likewise
              out_h = softmax(q_h k_h^T / sqrt(D)) v_h
  rep = concat_d-major(out_h) @ Wo + bo

Sharding: data-parallel over batch -> one batch element per NeuronCore,
zero collectives. All 8 cores run the same program (SPMD) on their own
batch slice.

Bilinear fusion (host-side, exact in f64): the q/k/v/output projections
collapse into two D x D matrices per head, cutting device FLOPs ~17%:
  scores   = (q Wq + bq)(k Wk + bk)^T
           = q (Wq Wk^T) k^T + [row-const terms] + bq (k Wk)^T
    row-constant terms are invariant under softmax over n -> dropped;
    GT_h = Wk_h Wq_h^T is the fused score matrix, and the column bias
    colb[n] = k_n . (Wk_h bq_h)/sqrt(D) folds into the exp activation's
    per-partition bias. bk drops out of the computation entirely.
  out Wo_h = A (v Wv + bv) Wo_h = A v (Wv Wo_h) + (softmax rows sum to 1)
    U_h = Wv_h Wo_h fuses the v and output projections; bv contributes
    the constant bv_h Wo_h, folded into the output bias on the host.

Per-core algorithm (transposed layouts, no on-chip transposes):
  kgT[g,n] = GT^T kT        lhsT=GT[d,g] rhs=kT[d,n]
  vu [n,e] = v @ U          lhsT=vT[d,n] rhs=U[d,e]
  colb[n-part, h] per-head  lhsT=kT[d,n-chunk] rhs=ub[d,h]  (column layout)
  ST [n,m] = kgT^T qT       lhsT=kgT[g,n] rhs=qT[g,m]   (q used raw!)
  ET = exp(ST/sqrt(D) + colb)  (no max subtraction: scores ~ N(0,1))
  R  [m]   = sum_n ET       (DVE chunk-accumulate + GpSimd partition_all_reduce)
  rep'[m,e] = ET^T vu       lhsT=ET[n,m-chunk] rhs=vu[n,e]  (direct [m,d]!)
  rep[m,e] += rep'/R        (softmax normalization deferred past the PV matmul;
                             R transposed to column layout via a tiny DRAM
                             round-trip, 1/R is a per-partition scalar)

Scheduling (TensorE measured ~99% busy inside its span):
  - score groups are emitted 2 n-chunk iterations ahead of the PV groups:
    the PV stationary operand IS the exp output, so without runway the
    exp latency would stall the PE every iteration;
  - the 1/R scaling runs on the DVE (tensor_scalar_mul), never ScalarE,
    so it can't delay the next block's first exp;
  - the lag-1 normalize/accumulate chain is emitted MID-inner-loop so its
    DVE ops interleave with the esum adds, and the post-loop drain only
    handles the final block (with DVE StreamTranspose replacing the DRAM
    round-trip since nothing hides its latency there);
  - psum evacuation copies are emitted right after the tail PV groups so
    the accumulator banks free before the next block's start=True writes;
  - the next head's kgT/vu projections are emitted at the end of each
    head's last block (independent PE work covering the softmax tail).

All matmul operands are bf16 (f32 PSUM accumulation): measured end-to-end
rel err vs the f32 reference is ~4e-3. fp8 (DoubleRow) was evaluated and
rejected: e4m3's 3-bit mantissa adds >=3.5e-2 rel err, over the 2e-2 gate.
"""

import numpy as np
import ml_dtypes

P = 128
B, N, M, D, H = 8, 2048, 2048, 512, 8


def build_program(n=N, m=M, d=D, h_cnt=H, mb=512):
    import concourse.bass as bass
    import concourse.tile as tile
    from concourse import bacc, bass_isa, mybir

    BF = mybir.dt.bfloat16
    F32 = mybir.dt.float32
    EXP = mybir.ActivationFunctionType.Exp

    DC = d // P        # contraction chunk count
    NCH = n // P       # n chunks
    NMB = m // mb      # m blocks
    MCL = mb // P      # m chunks per m block
    NF = n // mb       # n free-blocks for kgT projection
    inv_sqrt_d = float(d) ** -0.5

    nc = bacc.Bacc()
    kT = nc.declare_dram_parameter("kT", [d, n], BF, isOutput=False)
    vT = nc.declare_dram_parameter("vT", [d, n], BF, isOutput=False)
    qT = nc.declare_dram_parameter("qT", [d, m], BF, isOutput=False)
    # GT = Wk_h Wq_h^T (fused score matrix), U = Wv_h Wo_h (fused v/out proj)
    GT = nc.declare_dram_parameter("GT", [h_cnt, d, d], BF, isOutput=False)
    U = nc.declare_dram_parameter("U", [h_cnt, d, d], BF, isOutput=False)
    # ub[:, h] = (Wk_h bq_h) / sqrt(D): score column-bias generators
    ub = nc.declare_dram_parameter("ub", [d, h_cnt], BF, isOutput=False)
    # bo here is bo + sum_h bv[h] @ Wo_h (bv folded on host)
    bo = nc.declare_dram_parameter("bo", [d], F32, isOutput=False)
    out = nc.declare_dram_parameter("out", [m, d], F32, isOutput=True)

    with (
        tile.TileContext(nc) as tc,
        tc.tile_pool(name="constp", bufs=1) as constp,
        tc.tile_pool(name="inp", bufs=1) as inp,
        tc.tile_pool(name="wts", bufs=2) as wts,
        tc.tile_pool(name="proj", bufs=1) as proj,
        tc.tile_pool(name="etp", bufs=6) as etp,
        tc.tile_pool(name="esp", bufs=3) as esp,
        tc.tile_pool(name="otp", bufs=3) as otp,
        tc.tile_pool(name="rip", bufs=2) as rip,
        tc.tile_pool(name="ftp", bufs=3) as ftp,
        tc.tile_pool(name="accp", bufs=1) as accp,
        tc.tile_pool(name="drp", bufs=3, space="DRAM") as drp,
        tc.tile_pool(name="pst", bufs=3, space="PSUM") as pst,
        tc.tile_pool(name="pso", bufs=1, space="PSUM") as pso,
        tc.tile_pool(name="pcb", bufs=1, space="PSUM") as pcb,
    ):
        # resident transposed inputs [d-chunk partitions, chunk, seq]
        kT_sb = inp.tile([P, DC, n], BF, name="kT_sb", tag="kT")
        vT_sb = inp.tile([P, DC, n], BF, name="vT_sb", tag="vT")
        qT_sb = inp.tile([P, DC, m], BF, name="qT_sb", tag="qT")

        def load_w(h, w_dram, tag, split=1):
            w_sb = wts.tile([P, DC, d], BF, name=f"{tag}{h}", tag=tag)
            src = w_dram[h].rearrange("(c p) e -> c p e", p=P)
            step = d // split
            for dc in range(DC):
                for s in range(split):
                    nc.sync.dma_start(
                        out=w_sb[:, dc, s * step:(s + 1) * step],
                        in_=src[dc][:, s * step:(s + 1) * step],
                    )
            return w_sb

        def load_input(x_sb, x_dram, length, split0=1):
            # ~128KB pieces spread across DMA queues so the first consumers
            # aren't gated on one 512KB-per-queue transfer. split0 halves the
            # first n-block's pieces further: with all 16 queues draining in
            # parallel, smaller first pieces complete sooner, letting head 0's
            # kg projection start earlier.
            src = x_dram[:].rearrange("(c p) n -> c p n", p=P)
            for nf in range(length // mb):
                step = mb // (split0 if nf == 0 else 1)
                for dc in range(DC):
                    for s in range(mb // step):
                        lo = nf * mb + s * step
                        nc.sync.dma_start(
                            out=x_sb[:, dc, lo:lo + step],
                            in_=src[dc][:, lo:lo + step],
                        )

        def project_colb():
            # per-head score column-bias, computed directly in column layout:
            # colb[p, ncc, h] = sum_d kT[d, ncc*P+p] * ub[d, h]
            colb_sb = constp.tile([P, NCH, h_cnt], F32, name="colb", tag="colb")
            for ncc in range(NCH):
                cps = pcb.tile([P, h_cnt], F32, name=f"cb{ncc}", tag="cb")
                for dc in range(DC):
                    nc.tensor.matmul(
                        cps,
                        lhsT=kT_sb[:, dc, ncc * P:(ncc + 1) * P],
                        rhs=ub_sb[:, dc, :],
                        start=(dc == 0),
                        stop=(dc == DC - 1),
                    )
                nc.scalar.copy(out=colb_sb[:, ncc, :], in_=cps)
            return colb_sb

        def project_head(h, gt_sb, u_sb):
            # kgT[g, n], bf16 (no bias: colb rides the exp activation).
            # nf-major so head 0's projection can start as soon as the first
            # kT DMA pieces land instead of waiting for the full tensor.
            kgT_sb = proj.tile([P, DC, n], BF, name=f"kgT{h}", tag="kgT")
            for nf in range(NF):
                for ec in range(DC):
                    ps = pst.tile([P, mb], F32, name=f"psk{h}_{ec}_{nf}", tag="st")
                    for dc in range(DC):
                        nc.tensor.matmul(
                            ps,
                            lhsT=gt_sb[:, dc, ec * P:(ec + 1) * P],
                            rhs=kT_sb[:, dc, nf * mb:(nf + 1) * mb],
                            start=(dc == 0),
                            stop=(dc == DC - 1),
                        )
                    nc.scalar.copy(out=kgT_sb[:, ec, nf * mb:(nf + 1) * mb], in_=ps)
            # vu[n, e], bf16 (no bias: bv folded into bo on the host)
            vu_sb = proj.tile([P, NCH, d], BF, name=f"vu{h}", tag="vu")
            for ncc in range(NCH):
                ps = pst.tile([P, d], F32, name=f"psv{h}_{ncc}", tag="st")
                for dc in range(DC):
                    nc.tensor.matmul(
                        ps,
                        lhsT=vT_sb[:, dc, ncc * P:(ncc + 1) * P],
                        rhs=u_sb[:, dc, :],
                        start=(dc == 0),
                        stop=(dc == DC - 1),
                    )
                nc.scalar.copy(out=vu_sb[:, ncc, :], in_=ps)
            return kgT_sb, vu_sb

        def final_acc(h, mbi, ots, rcinv, drain=False):
            # rep accumulation of the UNNORMALIZED attention output: m is the
            # partition axis and 1/R is a per-partition scalar. Both ops run
            # on the DVE: a ScalarE mul here would sit in the ScalarE queue
            # between this block's exps and the next block's first exp, and
            # the next block's first PV matmul (lhsT=et) waits on that exp.
            # In the post-loop drain there are no exps left, so the muls go
            # to the (idle) ScalarE to overlap with the DVE adds.
            for mcl in range(MCL):
                mc = mbi * MCL + mcl
                tmp = ftp.tile([P, d], F32, name=f"ft{h}_{mbi}_{mcl}", tag="ft")
                if drain:
                    nc.scalar.mul(out=tmp, in_=ots[:, mcl, :], mul=rcinv[:, mcl:mcl + 1])
                else:
                    nc.vector.tensor_scalar_mul(tmp, ots[:, mcl, :], rcinv[:, mcl:mcl + 1])
                nc.vector.tensor_add(out=rep_sb[:, mc, :], in0=rep_sb[:, mc, :], in1=tmp)
                if h == h_cnt - 1:
                    nc.sync.dma_start(
                        out=out[:].rearrange("(c p) e -> c p e", p=P)[mc],
                        in_=rep_sb[:, mc, :],
                    )

        def normalize(st, drain=False):
            # softmax denominators, one block behind the attention loop.
            # r_rep holds R replicated across partitions (row layout, indexed
            # by m along the free axis); the division happens at the rep
            # accumulation where m is the PARTITION axis, so R must transpose
            # into column layout [128, MCL]. Steady state uses a tiny DRAM
            # round-trip (two chained DMAs, zero engine cost, latency hidden
            # by the one-block lag). The post-loop drain can't hide that ~4us
            # latency, so it instead assembles the columns with DVE 32x32
            # StreamTranspose blocks (rows are replicated, so blocks with
            # matching partition bases pick out rt[32a+i, mcl] = r[mcl*128+
            # 32a+i]); the DVE is idle in the drain.
            h, mbi, ots, r_rep = st
            if drain:
                rt = rip.tile([P, MCL, 32], F32, name=f"rt{h}_{mbi}", tag="rt", bufs=1)
                for a in range(4):
                    row = r_rep[32 * a:32 * (a + 1), :].rearrange("p (m x) -> p m x", x=P)
                    for mcl in range(MCL):
                        nc.vector.transpose(
                            out=rt[32 * a:32 * (a + 1), mcl, :],
                            in_=row[:, mcl, 32 * a:32 * (a + 1)],
                        )
                rcinv = rip.tile([P, MCL], F32, name=f"rci{h}_{mbi}", tag="rci", bufs=3)
                nc.vector.reciprocal(out=rcinv, in_=rt[:, :, 0])
                return (h, mbi, ots, rcinv)
            rdram = drp.tile([mb], F32, name=f"rd{h}_{mbi}", tag="rd")
            nc.sync.dma_start(out=rdram[:], in_=r_rep[0:1, :])
            rcol = rip.tile([P, MCL], F32, name=f"rc{h}_{mbi}", tag="rc", bufs=3)
            nc.sync.dma_start(out=rcol, in_=rdram[:].rearrange("(c p) -> p c", p=P))
            rcinv = rip.tile([P, MCL], F32, name=f"rci{h}_{mbi}", tag="rci", bufs=3)
            nc.vector.reciprocal(out=rcinv, in_=rcol)
            return (h, mbi, ots, rcinv)

        # head 0 loads: DMA issue order = first-use order. The critical-path
        # pieces (gt0 + kT) go first; bo/ub and the rep-init DVE copies are
        # deferred past the projection emission (first needed ~40us in).
        gt_cur = load_w(0, GT, "gt", split=2)
        load_input(kT_sb, kT, n, split0=2)
        u_cur = load_w(0, U, "u")
        load_input(vT_sb, vT, n)
        load_input(qT_sb, qT, m)
        kgT_cur, vu_cur = project_head(0, gt_cur, u_cur)

        ub_sb = constp.tile([P, DC, h_cnt], BF, name="ub_sb", tag="ub")
        nc.sync.dma_start(out=ub_sb, in_=ub[:].rearrange("(c p) h -> p c h", p=P))
        colb_sb = project_colb()

        bo_sb = constp.tile([P, d], F32, name="bo_sb", tag="bo")
        bo_ap = bo[:]
        nc.sync.dma_start(
            out=bo_sb,
            in_=bass.AP(tensor=bo_ap.tensor, offset=bo_ap.offset, ap=[[0, P], *bo_ap.ap]),
        )
        # rep accumulator, initialized with the (effective) output bias
        rep_sb = accp.tile([P, m // P, d], F32, name="rep_sb", tag="rep")
        for mc in range(m // P):
            nc.vector.tensor_copy(out=rep_sb[:, mc, :], in_=bo_sb)
        gt_next = u_next = kgT_next = vu_next = None
        pend_norm = None  # attention output awaiting softmax normalize
        pend_fp = None    # normalized output awaiting rep accumulation

        def emit_pv(h, mbi, rp_ps, et, ncc):
            for mcl in range(MCL):
                nc.tensor.matmul(
                    rp_ps[mcl],
                    lhsT=et[:, mcl * P:(mcl + 1) * P],
                    rhs=vu_cur[:, ncc, :],
                    start=(ncc == 0),
                    stop=(ncc == NCH - 1),
                )

        for h in range(h_cnt):
            for mbi in range(NMB):
                # ---- attention inner loop over n chunks ----
                # scores are emitted 2 iterations ahead of PV: the PV matmul's
                # stationary operand IS the exp output, so the TensorEngine
                # needs >= 2 score groups of runway for the ScalarE exp to
                # complete without exposing its latency
                rp_ps = [
                    pso.tile([P, d], F32, name=f"rp{h}_{mbi}_{mcl}", tag=f"rp{mcl}")
                    for mcl in range(MCL)
                ]
                esum = esp.tile([P, mb], F32, name=f"es{h}_{mbi}", tag="esum", bufs=3)
                ets = []
                for ncc in range(NCH):
                    st_ps = pst.tile([P, mb], F32, name=f"st{h}_{mbi}_{ncc}", tag="st")
                    for dc in range(DC):
                        nc.tensor.matmul(
                            st_ps,
                            lhsT=kgT_cur[:, dc, ncc * P:(ncc + 1) * P],
                            rhs=qT_sb[:, dc, mbi * mb:(mbi + 1) * mb],
                            start=(dc == 0),
                            stop=(dc == DC - 1),
                        )
                    et = etp.tile([P, mb], BF, name=f"et{h}_{mbi}_{ncc}", tag="et", bufs=6)
                    nc.scalar.activation(out=et, in_=st_ps, func=EXP,
                                         scale=inv_sqrt_d,
                                         bias=colb_sb[:, ncc, h:h + 1])
                    if ncc == 0:
                        nc.vector.tensor_copy(out=esum, in_=et)
                    else:
                        nc.vector.tensor_add(out=esum, in0=esum, in1=et)
                    ets.append(et)
                    if ncc >= 2:
                        emit_pv(h, mbi, rp_ps, ets[ncc - 2], ncc - 2)
                    # lag-1 softmax chain, emitted MID-loop so its DVE ops
                    # interleave with this block's esum adds instead of
                    # serializing after the last block (short pipeline drain).
                    # By ncc==5 the previous block's gpsimd row-sum reduce and
                    # the R-transpose DMA round-trip have long completed.
                    if ncc == 5 and pend_norm is not None:
                        pend_fp = normalize(pend_norm)
                        pend_norm = None
                    if ncc == 12 and pend_fp is not None:
                        final_acc(*pend_fp)
                        pend_fp = None
                emit_pv(h, mbi, rp_ps, ets[NCH - 2], NCH - 2)
                emit_pv(h, mbi, rp_ps, ets[NCH - 1], NCH - 1)

                # evacuate rep' psum to SBUF (unnormalized, f32) immediately:
                # frees the psum banks for the next block's PV groups so the
                # softmax chain can lag without holding the TensorEngine
                ots = otp.tile([P, MCL, d], F32, name=f"ots{h}_{mbi}", tag="ots")
                for mcl in range(MCL):
                    nc.vector.tensor_copy(out=ots[:, mcl, :], in_=rp_ps[mcl])

                # row sums on GpSimd (own FIFO, runs during the next block)
                r_rep = rip.tile([P, mb], F32, name=f"rr{h}_{mbi}", tag="rr", bufs=3)
                nc.gpsimd.partition_all_reduce(r_rep, esum[:], P, bass_isa.ReduceOp.add)

                # ---- lookahead emission: independent PE work ----
                if mbi == 0 and h + 1 < h_cnt:
                    gt_next = load_w(h + 1, GT, "gt")
                    u_next = load_w(h + 1, U, "u")
                if mbi == NMB - 1 and h + 1 < h_cnt:
                    kgT_next, vu_next = project_head(h + 1, gt_next, u_next)

                pend_norm = (h, mbi, ots, r_rep)

                if mbi == NMB - 1 and h + 1 < h_cnt:
                    kgT_cur, vu_cur = kgT_next, vu_next

        # drain: only the last block's softmax chain remains
        if pend_fp is not None:
            final_acc(*pend_fp, drain=True)
        if pend_norm is not None:
            final_acc(*normalize(pend_norm, drain=True), drain=True)

    if not nc.is_finalized():
        nc.finalize()
    return nc


def prepare_in_maps(k, v, q, Wk, bk, Wv, bv, Wq, bq, Wo, bo):
    """Shard + fuse + lay out the full inputs for the 8 cores (host numpy)."""
    bf16 = ml_dtypes.bfloat16
    f32 = np.float32
    f64 = np.float64
    h_cnt, d = Wk.shape[0], Wk.shape[1]
    # Wo rows are ordered d*H + h (d-major flatten): per-head slice h::H
    Wo_h = np.stack([Wo[h::h_cnt, :] for h in range(h_cnt)])  # [H, D, D]
    # fused score matrix and fused v/output projection (exact, f64)
    GT = np.stack([Wk[h].astype(f64) @ Wq[h].astype(f64).T for h in range(h_cnt)])
    U = np.stack([Wv[h].astype(f64) @ Wo_h[h].astype(f64) for h in range(h_cnt)])
    # score column-bias generators (bq term; bk and the row-constant terms
    # are invariant under softmax and dropped)
    ub = np.stack(
        [Wk[h].astype(f64) @ bq[h].astype(f64) for h in range(h_cnt)], axis=1
    ) * float(d) ** -0.5  # [D, H]
    # softmax rows sum to 1, so each head's bv contributes the constant
    # vector bv[h] @ U... fold all of it into the output bias
    bo_eff = bo.astype(f64) + sum(
        bv[h].astype(f64) @ Wo_h[h].astype(f64) for h in range(h_cnt)
    )
    shared = {
        "GT": np.ascontiguousarray(GT).astype(bf16),
        "U": np.ascontiguousarray(U).astype(bf16),
        "ub": np.ascontiguousarray(ub).astype(bf16),
        "bo": np.ascontiguousarray(bo_eff).astype(f32),
    }
    in_maps = []
    for b in range(k.shape[0]):
        in_maps.append({
            "kT": np.ascontiguousarray(k[b].T).astype(bf16),
            "vT": np.ascontiguousarray(v[b].T).astype(bf16),
            "qT": np.ascontiguousarray(q[b].T).astype(bf16),
            **shared,
        })
    return in_maps


def run(in_maps, trace=False):
    from concourse.bass_utils import run_bass_kernel_spmd

    nc = build_program()
    res = run_bass_kernel_spmd(nc, in_maps, core_ids=list(range(len(in_maps))), trace=trace)
    out = np.stack([np.asarray(r["out"], dtype=np.float32) for r in res.results])
    return out, res


def kernel(k, v, q, Wk, bk, Wv, bv, Wq, bq, Wo, bo):
    args = [np.asarray(a) for a in (k, v, q, Wk, bk, Wv, bv, Wq, bq, Wo, bo)]
    in_maps = prepare_in_maps(*args)
    out, _ = run(in_maps, trace=False)
    return out


# revision 23
# speedup vs baseline: 1.0055x; 1.0055x over previous
"""Trainium2 Bass kernel for nn_Attention (dense multi-head cross-attention).

Problem: B=8 batches, N=M=2048 seq, D=512 hidden, H=8 heads.
  per head h: k_h = k @ Wk[h] + bk[h]; v_h, q_h likewise
              out_h = softmax(q_h k_h^T / sqrt(D)) v_h
  rep = concat_d-major(out_h) @ Wo + bo

Sharding: data-parallel over batch -> one batch element per NeuronCore,
zero collectives. All 8 cores run the same program (SPMD) on their own
batch slice.

Bilinear fusion (host-side, exact in f64): the q/k/v/output projections
collapse into two D x D matrices per head, cutting device FLOPs ~17%:
  scores   = (q Wq + bq)(k Wk + bk)^T
           = q (Wq Wk^T) k^T + [row-const terms] + bq (k Wk)^T
    row-constant terms are invariant under softmax over n -> dropped;
    GT_h = Wk_h Wq_h^T is the fused score matrix, and the column bias
    colb[n] = k_n . (Wk_h bq_h)/sqrt(D) folds into the exp activation's
    per-partition bias. bk drops out of the computation entirely.
  out Wo_h = A (v Wv + bv) Wo_h = A v (Wv Wo_h) + (softmax rows sum to 1)
    U_h = Wv_h Wo_h fuses the v and output projections; bv contributes
    the constant bv_h Wo_h, folded into the output bias on the host.

Per-core algorithm (transposed layouts, no on-chip transposes):
  kgT[g,n] = GT^T kT        lhsT=GT[d,g] rhs=kT[d,n]
  vu [n,e] = v @ U          lhsT=vT[d,n] rhs=U[d,e]
  colb[n-part, h] per-head  lhsT=kT[d,n-chunk] rhs=ub[d,h]  (column layout)
  ST [n,m] = kgT^T qT       lhsT=kgT[g,n] rhs=qT[g,m]   (q used raw!)
  ET = exp(ST/sqrt(D) + colb)  (no max subtraction: scores ~ N(0,1))
  R  [m]   = sum_n ET       (DVE chunk-accumulate + GpSimd partition_all_reduce)
  rep'[m,e] = ET^T vu       lhsT=ET[n,m-chunk] rhs=vu[n,e]  (direct [m,d]!)
  rep[m,e] += rep'/R        (softmax normalization deferred past the PV matmul;
                             R transposed to column layout via a tiny DRAM
                             round-trip, 1/R is a per-partition scalar)

Scheduling (TensorE measured ~99% busy inside its span):
  - score groups are emitted 2 n-chunk iterations ahead of the PV groups:
    the PV stationary operand IS the exp output, so without runway the
    exp latency would stall the PE every iteration;
  - the 1/R scaling runs on the DVE (tensor_scalar_mul), never ScalarE,
    so it can't delay the next block's first exp;
  - the lag-1 normalize/accumulate chain is emitted MID-inner-loop so its
    DVE ops interleave with the esum adds, and the post-loop drain only
    handles the final block (with DVE StreamTranspose replacing the DRAM
    round-trip since nothing hides its latency there);
  - psum evacuation copies are emitted right after the tail PV groups so
    the accumulator banks free before the next block's start=True writes;
  - the next head's kgT/vu projections are emitted at the end of each
    head's last block (independent PE work covering the softmax tail).

All matmul operands are bf16 (f32 PSUM accumulation): measured end-to-end
rel err vs the f32 reference is ~4e-3. fp8 (DoubleRow) was evaluated and
rejected: e4m3's 3-bit mantissa adds >=3.5e-2 rel err, over the 2e-2 gate.
"""

import numpy as np
import ml_dtypes

P = 128
B, N, M, D, H = 8, 2048, 2048, 512, 8


def build_program(n=N, m=M, d=D, h_cnt=H, mb=512):
    import concourse.bass as bass
    import concourse.tile as tile
    from concourse import bacc, bass_isa, mybir

    BF = mybir.dt.bfloat16
    F32 = mybir.dt.float32
    EXP = mybir.ActivationFunctionType.Exp

    DC = d // P        # contraction chunk count
    NCH = n // P       # n chunks
    NMB = m // mb      # m blocks
    MCL = mb // P      # m chunks per m block
    NF = n // mb       # n free-blocks for kgT projection
    inv_sqrt_d = float(d) ** -0.5

    nc = bacc.Bacc()
    kT = nc.declare_dram_parameter("kT", [d, n], BF, isOutput=False)
    vT = nc.declare_dram_parameter("vT", [d, n], BF, isOutput=False)
    qT = nc.declare_dram_parameter("qT", [d, m], BF, isOutput=False)
    # GT = Wk_h Wq_h^T (fused score matrix), U = Wv_h Wo_h (fused v/out proj)
    GT = nc.declare_dram_parameter("GT", [h_cnt, d, d], BF, isOutput=False)
    U = nc.declare_dram_parameter("U", [h_cnt, d, d], BF, isOutput=False)
    # ub[:, h] = (Wk_h bq_h) / sqrt(D): score column-bias generators
    ub = nc.declare_dram_parameter("ub", [d, h_cnt], BF, isOutput=False)
    # bo here is bo + sum_h bv[h] @ Wo_h (bv folded on host)
    bo = nc.declare_dram_parameter("bo", [d], F32, isOutput=False)
    out = nc.declare_dram_parameter("out", [m, d], F32, isOutput=True)

    with (
        tile.TileContext(nc) as tc,
        tc.tile_pool(name="constp", bufs=1) as constp,
        tc.tile_pool(name="inp", bufs=1) as inp,
        tc.tile_pool(name="wts", bufs=2) as wts,
        tc.tile_pool(name="proj", bufs=1) as proj,
        tc.tile_pool(name="etp", bufs=6) as etp,
        tc.tile_pool(name="esp", bufs=3) as esp,
        tc.tile_pool(name="otp", bufs=3) as otp,
        tc.tile_pool(name="rip", bufs=2) as rip,
        tc.tile_pool(name="ftp", bufs=3) as ftp,
        tc.tile_pool(name="accp", bufs=1) as accp,
        tc.tile_pool(name="drp", bufs=3, space="DRAM") as drp,
        tc.tile_pool(name="pst", bufs=3, space="PSUM") as pst,
        tc.tile_pool(name="pso", bufs=1, space="PSUM") as pso,
        tc.tile_pool(name="pcb", bufs=1, space="PSUM") as pcb,
    ):
        # resident transposed inputs [d-chunk partitions, chunk, seq]
        kT_sb = inp.tile([P, DC, n], BF, name="kT_sb", tag="kT")
        vT_sb = inp.tile([P, DC, n], BF, name="vT_sb", tag="vT")
        qT_sb = inp.tile([P, DC, m], BF, name="qT_sb", tag="qT")

        def load_w(h, w_dram, tag, split=1):
            w_sb = wts.tile([P, DC, d], BF, name=f"{tag}{h}", tag=tag)
            src = w_dram[h].rearrange("(c p) e -> c p e", p=P)
            step = d // split
            for dc in range(DC):
                for s in range(split):
                    nc.sync.dma_start(
                        out=w_sb[:, dc, s * step:(s + 1) * step],
                        in_=src[dc][:, s * step:(s + 1) * step],
                    )
            return w_sb

        def load_input(x_sb, x_dram, length, split0=1):
            # ~128KB pieces spread across DMA queues so the first consumers
            # aren't gated on one 512KB-per-queue transfer. split0 halves the
            # first n-block's pieces further: with all 16 queues draining in
            # parallel, smaller first pieces complete sooner, letting head 0's
            # kg projection start earlier.
            src = x_dram[:].rearrange("(c p) n -> c p n", p=P)
            for nf in range(length // mb):
                step = mb // (split0 if nf == 0 else 1)
                for dc in range(DC):
                    for s in range(mb // step):
                        lo = nf * mb + s * step
                        nc.sync.dma_start(
                            out=x_sb[:, dc, lo:lo + step],
                            in_=src[dc][:, lo:lo + step],
                        )

        def project_colb():
            # per-head score column-bias, computed directly in column layout:
            # colb[p, ncc, h] = sum_d kT[d, ncc*P+p] * ub[d, h]
            colb_sb = constp.tile([P, NCH, h_cnt], F32, name="colb", tag="colb")
            for ncc in range(NCH):
                cps = pcb.tile([P, h_cnt], F32, name=f"cb{ncc}", tag="cb")
                for dc in range(DC):
                    nc.tensor.matmul(
                        cps,
                        lhsT=kT_sb[:, dc, ncc * P:(ncc + 1) * P],
                        rhs=ub_sb[:, dc, :],
                        start=(dc == 0),
                        stop=(dc == DC - 1),
                    )
                nc.scalar.copy(out=colb_sb[:, ncc, :], in_=cps)
            return colb_sb

        def project_head(h, gt_sb, u_sb):
            # kgT[g, n], bf16 (no bias: colb rides the exp activation).
            # nf-major so head 0's projection can start as soon as the first
            # kT DMA pieces land instead of waiting for the full tensor.
            kgT_sb = proj.tile([P, DC, n], BF, name=f"kgT{h}", tag="kgT")
            for nf in range(NF):
                for ec in range(DC):
                    ps = pst.tile([P, mb], F32, name=f"psk{h}_{ec}_{nf}", tag="st")
                    for dc in range(DC):
                        nc.tensor.matmul(
                            ps,
                            lhsT=gt_sb[:, dc, ec * P:(ec + 1) * P],
                            rhs=kT_sb[:, dc, nf * mb:(nf + 1) * mb],
                            start=(dc == 0),
                            stop=(dc == DC - 1),
                        )
                    nc.scalar.copy(out=kgT_sb[:, ec, nf * mb:(nf + 1) * mb], in_=ps)
            # vu[n, e], bf16 (no bias: bv folded into bo on the host)
            vu_sb = proj.tile([P, NCH, d], BF, name=f"vu{h}", tag="vu")
            for ncc in range(NCH):
                ps = pst.tile([P, d], F32, name=f"psv{h}_{ncc}", tag="st")
                for dc in range(DC):
                    nc.tensor.matmul(
                        ps,
                        lhsT=vT_sb[:, dc, ncc * P:(ncc + 1) * P],
                        rhs=u_sb[:, dc, :],
                        start=(dc == 0),
                        stop=(dc == DC - 1),
                    )
                nc.scalar.copy(out=vu_sb[:, ncc, :], in_=ps)
            return kgT_sb, vu_sb

        def final_acc(h, mbi, ots, rcinv, drain=False):
            # rep accumulation of the UNNORMALIZED attention output: m is the
            # partition axis and 1/R is a per-partition scalar. Both ops run
            # on the DVE: a ScalarE mul here would sit in the ScalarE queue
            # between this block's exps and the next block's first exp, and
            # the next block's first PV matmul (lhsT=et) waits on that exp.
            # In the post-loop drain there are no exps left, so the muls go
            # to the (idle) ScalarE to overlap with the DVE adds.
            for mcl in range(MCL):
                mc = mbi * MCL + mcl
                tmp = ftp.tile([P, d], F32, name=f"ft{h}_{mbi}_{mcl}", tag="ft")
                if drain:
                    nc.scalar.mul(out=tmp, in_=ots[:, mcl, :], mul=rcinv[:, mcl:mcl + 1])
                else:
                    nc.vector.tensor_scalar_mul(tmp, ots[:, mcl, :], rcinv[:, mcl:mcl + 1])
                nc.vector.tensor_add(out=rep_sb[:, mc, :], in0=rep_sb[:, mc, :], in1=tmp)
                if h == h_cnt - 1:
                    nc.sync.dma_start(
                        out=out[:].rearrange("(c p) e -> c p e", p=P)[mc],
                        in_=rep_sb[:, mc, :],
                    )

        def normalize(st, drain=False):
            # softmax denominators, one block behind the attention loop.
            # r_rep holds R replicated across partitions (row layout, indexed
            # by m along the free axis); the division happens at the rep
            # accumulation where m is the PARTITION axis, so R must transpose
            # into column layout [128, MCL]. Steady state uses a tiny DRAM
            # round-trip (two chained DMAs, zero engine cost, latency hidden
            # by the one-block lag). The post-loop drain can't hide that ~4us
            # latency, so it instead assembles the columns with DVE 32x32
            # StreamTranspose blocks (rows are replicated, so blocks with
            # matching partition bases pick out rt[32a+i, mcl] = r[mcl*128+
            # 32a+i]); the DVE is idle in the drain.
            h, mbi, ots, r_rep = st
            if drain:
                rt = rip.tile([P, MCL, 32], F32, name=f"rt{h}_{mbi}", tag="rt", bufs=1)
                for a in range(4):
                    row = r_rep[32 * a:32 * (a + 1), :].rearrange("p (m x) -> p m x", x=P)
                    for mcl in range(MCL):
                        nc.vector.transpose(
                            out=rt[32 * a:32 * (a + 1), mcl, :],
                            in_=row[:, mcl, 32 * a:32 * (a + 1)],
                        )
                rcinv = rip.tile([P, MCL], F32, name=f"rci{h}_{mbi}", tag="rci", bufs=3)
                nc.vector.reciprocal(out=rcinv, in_=rt[:, :, 0])
                return (h, mbi, ots, rcinv)
            rdram = drp.tile([mb], F32, name=f"rd{h}_{mbi}", tag="rd")
            nc.sync.dma_start(out=rdram[:], in_=r_rep[0:1, :])
            rcol = rip.tile([P, MCL], F32, name=f"rc{h}_{mbi}", tag="rc", bufs=3)
            nc.sync.dma_start(out=rcol, in_=rdram[:].rearrange("(c p) -> p c", p=P))
            rcinv = rip.tile([P, MCL], F32, name=f"rci{h}_{mbi}", tag="rci", bufs=3)
            nc.vector.reciprocal(out=rcinv, in_=rcol)
            return (h, mbi, ots, rcinv)

        # head 0 loads: DMA issue order = first-use order. The critical-path
        # pieces (gt0 + kT) go first; bo/ub and the rep-init DVE copies are
        # deferred past the projection emission (first needed ~40us in).
        gt_cur = load_w(0, GT, "gt")
        load_input(kT_sb, kT, n)
        u_cur = load_w(0, U, "u")
        load_input(vT_sb, vT, n)
        load_input(qT_sb, qT, m)
        kgT_cur, vu_cur = project_head(0, gt_cur, u_cur)

        ub_sb = constp.tile([P, DC, h_cnt], BF, name="ub_sb", tag="ub")
        nc.sync.dma_start(out=ub_sb, in_=ub[:].rearrange("(c p) h -> p c h", p=P))
        colb_sb = project_colb()

        bo_sb = constp.tile([P, d], F32, name="bo_sb", tag="bo")
        bo_ap = bo[:]
        nc.sync.dma_start(
            out=bo_sb,
            in_=bass.AP(tensor=bo_ap.tensor, offset=bo_ap.offset, ap=[[0, P], *bo_ap.ap]),
        )
        # rep accumulator, initialized with the (effective) output bias
        rep_sb = accp.tile([P, m // P, d], F32, name="rep_sb", tag="rep")
        for mc in range(m // P):
            nc.vector.tensor_copy(out=rep_sb[:, mc, :], in_=bo_sb)
        gt_next = u_next = kgT_next = vu_next = None
        pend_norm = None  # attention output awaiting softmax normalize
        pend_fp = None    # normalized output awaiting rep accumulation

        def emit_pv(h, mbi, rp_ps, et, ncc):
            for mcl in range(MCL):
                nc.tensor.matmul(
                    rp_ps[mcl],
                    lhsT=et[:, mcl * P:(mcl + 1) * P],
                    rhs=vu_cur[:, ncc, :],
                    start=(ncc == 0),
                    stop=(ncc == NCH - 1),
                )

        for h in range(h_cnt):
            for mbi in range(NMB):
                # ---- attention inner loop over n chunks ----
                # scores are emitted 2 iterations ahead of PV: the PV matmul's
                # stationary operand IS the exp output, so the TensorEngine
                # needs >= 2 score groups of runway for the ScalarE exp to
                # complete without exposing its latency
                rp_ps = [
                    pso.tile([P, d], F32, name=f"rp{h}_{mbi}_{mcl}", tag=f"rp{mcl}")
                    for mcl in range(MCL)
                ]
                esum = esp.tile([P, mb], F32, name=f"es{h}_{mbi}", tag="esum", bufs=3)
                ets = []
                for ncc in range(NCH):
                    st_ps = pst.tile([P, mb], F32, name=f"st{h}_{mbi}_{ncc}", tag="st")
                    for dc in range(DC):
                        nc.tensor.matmul(
                            st_ps,
                            lhsT=kgT_cur[:, dc, ncc * P:(ncc + 1) * P],
                            rhs=qT_sb[:, dc, mbi * mb:(mbi + 1) * mb],
                            start=(dc == 0),
                            stop=(dc == DC - 1),
                        )
                    et = etp.tile([P, mb], BF, name=f"et{h}_{mbi}_{ncc}", tag="et", bufs=6)
                    nc.scalar.activation(out=et, in_=st_ps, func=EXP,
                                         scale=inv_sqrt_d,
                                         bias=colb_sb[:, ncc, h:h + 1])
                    if ncc == 0:
                        nc.vector.tensor_copy(out=esum, in_=et)
                    else:
                        nc.vector.tensor_add(out=esum, in0=esum, in1=et)
                    ets.append(et)
                    if ncc >= 2:
                        emit_pv(h, mbi, rp_ps, ets[ncc - 2], ncc - 2)
                    # lag-1 softmax chain, emitted MID-loop so its DVE ops
                    # interleave with this block's esum adds instead of
                    # serializing after the last block (short pipeline drain).
                    # By ncc==5 the previous block's gpsimd row-sum reduce and
                    # the R-transpose DMA round-trip have long completed.
                    if ncc == 5 and pend_norm is not None:
                        pend_fp = normalize(pend_norm)
                        pend_norm = None
                    if ncc == 12 and pend_fp is not None:
                        final_acc(*pend_fp)
                        pend_fp = None
                emit_pv(h, mbi, rp_ps, ets[NCH - 2], NCH - 2)
                emit_pv(h, mbi, rp_ps, ets[NCH - 1], NCH - 1)

                # evacuate rep' psum to SBUF (unnormalized, f32) immediately:
                # frees the psum banks for the next block's PV groups so the
                # softmax chain can lag without holding the TensorEngine
                ots = otp.tile([P, MCL, d], F32, name=f"ots{h}_{mbi}", tag="ots")
                for mcl in range(MCL):
                    nc.vector.tensor_copy(out=ots[:, mcl, :], in_=rp_ps[mcl])

                # row sums on GpSimd (own FIFO, runs during the next block)
                r_rep = rip.tile([P, mb], F32, name=f"rr{h}_{mbi}", tag="rr", bufs=3)
                nc.gpsimd.partition_all_reduce(r_rep, esum[:], P, bass_isa.ReduceOp.add)

                # ---- lookahead emission: independent PE work ----
                if mbi == 0 and h + 1 < h_cnt:
                    gt_next = load_w(h + 1, GT, "gt")
                    u_next = load_w(h + 1, U, "u")
                if mbi == NMB - 1 and h + 1 < h_cnt:
                    kgT_next, vu_next = project_head(h + 1, gt_next, u_next)

                pend_norm = (h, mbi, ots, r_rep)

                if mbi == NMB - 1 and h + 1 < h_cnt:
                    kgT_cur, vu_cur = kgT_next, vu_next

        # drain: only the last block's softmax chain remains
        if pend_fp is not None:
            final_acc(*pend_fp, drain=True)
        if pend_norm is not None:
            final_acc(*normalize(pend_norm, drain=True), drain=True)

    if not nc.is_finalized():
        nc.finalize()
    return nc


def prepare_in_maps(k, v, q, Wk, bk, Wv, bv, Wq, bq, Wo, bo):
    """Shard + fuse + lay out the full inputs for the 8 cores (host numpy)."""
    bf16 = ml_dtypes.bfloat16
    f32 = np.float32
    f64 = np.float64
    h_cnt, d = Wk.shape[0], Wk.shape[1]
    # Wo rows are ordered d*H + h (d-major flatten): per-head slice h::H
    Wo_h = np.stack([Wo[h::h_cnt, :] for h in range(h_cnt)])  # [H, D, D]
    # fused score matrix and fused v/output projection (exact, f64)
    GT = np.stack([Wk[h].astype(f64) @ Wq[h].astype(f64).T for h in range(h_cnt)])
    U = np.stack([Wv[h].astype(f64) @ Wo_h[h].astype(f64) for h in range(h_cnt)])
    # score column-bias generators (bq term; bk and the row-constant terms
    # are invariant under softmax and dropped)
    ub = np.stack(
        [Wk[h].astype(f64) @ bq[h].astype(f64) for h in range(h_cnt)], axis=1
    ) * float(d) ** -0.5  # [D, H]
    # softmax rows sum to 1, so each head's bv contributes the constant
    # vector bv[h] @ U... fold all of it into the output bias
    bo_eff = bo.astype(f64) + sum(
        bv[h].astype(f64) @ Wo_h[h].astype(f64) for h in range(h_cnt)
    )
    shared = {
        "GT": np.ascontiguousarray(GT).astype(bf16),
        "U": np.ascontiguousarray(U).astype(bf16),
        "ub": np.ascontiguousarray(ub).astype(bf16),
        "bo": np.ascontiguousarray(bo_eff).astype(f32),
    }
    in_maps = []
    for b in range(k.shape[0]):
        in_maps.append({
            "kT": np.ascontiguousarray(k[b].T).astype(bf16),
            "vT": np.ascontiguousarray(v[b].T).astype(bf16),
            "qT": np.ascontiguousarray(q[b].T).astype(bf16),
            **shared,
        })
    return in_maps


def run(in_maps, trace=False):
    from concourse.bass_utils import run_bass_kernel_spmd

    nc = build_program()
    res = run_bass_kernel_spmd(nc, in_maps, core_ids=list(range(len(in_maps))), trace=trace)
    out = np.stack([np.asarray(r["out"], dtype=np.float32) for r in res.results])
    return out, res


def kernel(k, v, q, Wk, bk, Wv, bv, Wq, bq, Wo, bo):
    args = [np.asarray(a) for a in (k, v, q, Wk, bk, Wv, bv, Wq, bq, Wo, bo)]
    in_maps = prepare_in_maps(*args)
    out, _ = run(in_maps, trace=False)
    return out


# revision 24
# speedup vs baseline: 1.0084x; 1.0028x over previous
"""Trainium2 Bass kernel for nn_Attention (dense multi-head cross-attention).

Problem: B=8 batches, N=M=2048 seq, D=512 hidden, H=8 heads.
  per head h: k_h = k @ Wk[h] + bk[h]; v_h, q_h likewise
              out_h = softmax(q_h k_h^T / sqrt(D)) v_h
  rep = concat_d-major(out_h) @ Wo + bo

Sharding: data-parallel over batch -> one batch element per NeuronCore,
zero collectives. All 8 cores run the same program (SPMD) on their own
batch slice.

Bilinear fusion (host-side, exact in f64): the q/k/v/output projections
collapse into two D x D matrices per head, cutting device FLOPs ~17%:
  scores   = (q Wq + bq)(k Wk + bk)^T
           = q (Wq Wk^T) k^T + [row-const terms] + bq (k Wk)^T
    row-constant terms are invariant under softmax over n -> dropped;
    GT_h = Wk_h Wq_h^T is the fused score matrix, and the column bias
    colb[n] = k_n . (Wk_h bq_h)/sqrt(D) folds into the exp activation's
    per-partition bias. bk drops out of the computation entirely.
  out Wo_h = A (v Wv + bv) Wo_h = A v (Wv Wo_h) + (softmax rows sum to 1)
    U_h = Wv_h Wo_h fuses the v and output projections; bv contributes
    the constant bv_h Wo_h, folded into the output bias on the host.

Per-core algorithm (transposed layouts, no on-chip transposes):
  kgT[g,n] = GT^T kT        lhsT=GT[d,g] rhs=kT[d,n]
  vu [n,e] = v @ U          lhsT=vT[d,n] rhs=U[d,e]
  colb[n-part, h] per-head  lhsT=kT[d,n-chunk] rhs=ub[d,h]  (column layout)
  ST [n,m] = kgT^T qT       lhsT=kgT[g,n] rhs=qT[g,m]   (q used raw!)
  ET = exp(ST/sqrt(D) + colb)  (no max subtraction: scores ~ N(0,1))
  R  [m]   = sum_n ET       (DVE chunk-accumulate + GpSimd partition_all_reduce)
  rep'[m,e] = ET^T vu       lhsT=ET[n,m-chunk] rhs=vu[n,e]  (direct [m,d]!)
  rep[m,e] += rep'/R        (softmax normalization deferred past the PV matmul;
                             R transposed to column layout via a tiny DRAM
                             round-trip, 1/R is a per-partition scalar)

Scheduling (TensorE measured ~99% busy inside its span):
  - score groups are emitted 2 n-chunk iterations ahead of the PV groups:
    the PV stationary operand IS the exp output, so without runway the
    exp latency would stall the PE every iteration;
  - the 1/R scaling runs on the DVE (tensor_scalar_mul), never ScalarE,
    so it can't delay the next block's first exp;
  - the lag-1 normalize/accumulate chain is emitted MID-inner-loop so its
    DVE ops interleave with the esum adds, and the post-loop drain only
    handles the final block (with DVE StreamTranspose replacing the DRAM
    round-trip since nothing hides its latency there);
  - psum evacuation copies are emitted right after the tail PV groups so
    the accumulator banks free before the next block's start=True writes;
  - the next head's kgT/vu projections are emitted at the end of each
    head's last block (independent PE work covering the softmax tail).

All matmul operands are bf16 (f32 PSUM accumulation): measured end-to-end
rel err vs the f32 reference is ~4e-3. fp8 (DoubleRow) was evaluated and
rejected: e4m3's 3-bit mantissa adds >=3.5e-2 rel err, over the 2e-2 gate.
"""

import numpy as np
import ml_dtypes

P = 128
B, N, M, D, H = 8, 2048, 2048, 512, 8


def build_program(n=N, m=M, d=D, h_cnt=H, mb=512):
    import concourse.bass as bass
    import concourse.tile as tile
    from concourse import bacc, bass_isa, mybir

    BF = mybir.dt.bfloat16
    F32 = mybir.dt.float32
    EXP = mybir.ActivationFunctionType.Exp

    DC = d // P        # contraction chunk count
    NCH = n // P       # n chunks
    NMB = m // mb      # m blocks
    MCL = mb // P      # m chunks per m block
    NF = n // mb       # n free-blocks for kgT projection
    inv_sqrt_d = float(d) ** -0.5

    nc = bacc.Bacc()
    kT = nc.declare_dram_parameter("kT", [d, n], BF, isOutput=False)
    vT = nc.declare_dram_parameter("vT", [d, n], BF, isOutput=False)
    qT = nc.declare_dram_parameter("qT", [d, m], BF, isOutput=False)
    # GT = Wk_h Wq_h^T (fused score matrix), U = Wv_h Wo_h (fused v/out proj)
    GT = nc.declare_dram_parameter("GT", [h_cnt, d, d], BF, isOutput=False)
    U = nc.declare_dram_parameter("U", [h_cnt, d, d], BF, isOutput=False)
    # ub[:, h] = (Wk_h bq_h) / sqrt(D): score column-bias generators
    ub = nc.declare_dram_parameter("ub", [d, h_cnt], BF, isOutput=False)
    # bo here is bo + sum_h bv[h] @ Wo_h (bv folded on host)
    bo = nc.declare_dram_parameter("bo", [d], F32, isOutput=False)
    out = nc.declare_dram_parameter("out", [m, d], F32, isOutput=True)

    with (
        tile.TileContext(nc) as tc,
        tc.tile_pool(name="constp", bufs=1) as constp,
        tc.tile_pool(name="inp", bufs=1) as inp,
        tc.tile_pool(name="wts", bufs=2) as wts,
        tc.tile_pool(name="proj", bufs=1) as proj,
        tc.tile_pool(name="etp", bufs=6) as etp,
        tc.tile_pool(name="esp", bufs=3) as esp,
        tc.tile_pool(name="otp", bufs=3) as otp,
        tc.tile_pool(name="rip", bufs=2) as rip,
        tc.tile_pool(name="ftp", bufs=3) as ftp,
        tc.tile_pool(name="accp", bufs=1) as accp,
        tc.tile_pool(name="drp", bufs=3, space="DRAM") as drp,
        tc.tile_pool(name="pst", bufs=3, space="PSUM") as pst,
        tc.tile_pool(name="pso", bufs=1, space="PSUM") as pso,
        tc.tile_pool(name="pcb", bufs=1, space="PSUM") as pcb,
    ):
        # resident transposed inputs [d-chunk partitions, chunk, seq]
        kT_sb = inp.tile([P, DC, n], BF, name="kT_sb", tag="kT")
        vT_sb = inp.tile([P, DC, n], BF, name="vT_sb", tag="vT")
        qT_sb = inp.tile([P, DC, m], BF, name="qT_sb", tag="qT")

        def load_w(h, w_dram, tag, split=1):
            w_sb = wts.tile([P, DC, d], BF, name=f"{tag}{h}", tag=tag)
            src = w_dram[h].rearrange("(c p) e -> c p e", p=P)
            step = d // split
            for dc in range(DC):
                for s in range(split):
                    nc.sync.dma_start(
                        out=w_sb[:, dc, s * step:(s + 1) * step],
                        in_=src[dc][:, s * step:(s + 1) * step],
                    )
            return w_sb

        def load_input(x_sb, x_dram, length, split0=1):
            # ~128KB pieces spread across DMA queues so the first consumers
            # aren't gated on one 512KB-per-queue transfer. split0 halves the
            # first n-block's pieces further: with all 16 queues draining in
            # parallel, smaller first pieces complete sooner, letting head 0's
            # kg projection start earlier.
            src = x_dram[:].rearrange("(c p) n -> c p n", p=P)
            for nf in range(length // mb):
                step = mb // (split0 if nf == 0 else 1)
                for dc in range(DC):
                    for s in range(mb // step):
                        lo = nf * mb + s * step
                        nc.sync.dma_start(
                            out=x_sb[:, dc, lo:lo + step],
                            in_=src[dc][:, lo:lo + step],
                        )

        def project_colb():
            # per-head score column-bias, computed directly in column layout:
            # colb[p, ncc, h] = sum_d kT[d, ncc*P+p] * ub[d, h]
            colb_sb = constp.tile([P, NCH, h_cnt], F32, name="colb", tag="colb")
            for ncc in range(NCH):
                cps = pcb.tile([P, h_cnt], F32, name=f"cb{ncc}", tag="cb")
                for dc in range(DC):
                    nc.tensor.matmul(
                        cps,
                        lhsT=kT_sb[:, dc, ncc * P:(ncc + 1) * P],
                        rhs=ub_sb[:, dc, :],
                        start=(dc == 0),
                        stop=(dc == DC - 1),
                    )
                nc.scalar.copy(out=colb_sb[:, ncc, :], in_=cps)
            return colb_sb

        def project_head(h, gt_sb, u_sb):
            # kgT[g, n], bf16 (no bias: colb rides the exp activation).
            # nf-major so head 0's projection can start as soon as the first
            # kT DMA pieces land instead of waiting for the full tensor.
            kgT_sb = proj.tile([P, DC, n], BF, name=f"kgT{h}", tag="kgT")
            for nf in range(NF):
                for ec in range(DC):
                    ps = pst.tile([P, mb], F32, name=f"psk{h}_{ec}_{nf}", tag="st")
                    for dc in range(DC):
                        nc.tensor.matmul(
                            ps,
                            lhsT=gt_sb[:, dc, ec * P:(ec + 1) * P],
                            rhs=kT_sb[:, dc, nf * mb:(nf + 1) * mb],
                            start=(dc == 0),
                            stop=(dc == DC - 1),
                        )
                    nc.scalar.copy(out=kgT_sb[:, ec, nf * mb:(nf + 1) * mb], in_=ps)
            # vu[n, e], bf16 (no bias: bv folded into bo on the host)
            vu_sb = proj.tile([P, NCH, d], BF, name=f"vu{h}", tag="vu")
            for ncc in range(NCH):
                ps = pst.tile([P, d], F32, name=f"psv{h}_{ncc}", tag="st")
                for dc in range(DC):
                    nc.tensor.matmul(
                        ps,
                        lhsT=vT_sb[:, dc, ncc * P:(ncc + 1) * P],
                        rhs=u_sb[:, dc, :],
                        start=(dc == 0),
                        stop=(dc == DC - 1),
                    )
                nc.scalar.copy(out=vu_sb[:, ncc, :], in_=ps)
            return kgT_sb, vu_sb

        def final_acc(h, mbi, ots, rcinv, drain=False):
            # rep accumulation of the UNNORMALIZED attention output: m is the
            # partition axis and 1/R is a per-partition scalar. Both ops run
            # on the DVE: a ScalarE mul here would sit in the ScalarE queue
            # between this block's exps and the next block's first exp, and
            # the next block's first PV matmul (lhsT=et) waits on that exp.
            # In the post-loop drain there are no exps left, so the muls go
            # to the (idle) ScalarE to overlap with the DVE adds.
            for mcl in range(MCL):
                mc = mbi * MCL + mcl
                tmp = ftp.tile([P, d], F32, name=f"ft{h}_{mbi}_{mcl}", tag="ft")
                if drain:
                    nc.scalar.mul(out=tmp, in_=ots[:, mcl, :], mul=rcinv[:, mcl:mcl + 1])
                else:
                    nc.vector.tensor_scalar_mul(tmp, ots[:, mcl, :], rcinv[:, mcl:mcl + 1])
                nc.vector.tensor_add(out=rep_sb[:, mc, :], in0=rep_sb[:, mc, :], in1=tmp)
                if h == h_cnt - 1:
                    nc.sync.dma_start(
                        out=out[:].rearrange("(c p) e -> c p e", p=P)[mc],
                        in_=rep_sb[:, mc, :],
                    )

        def normalize(st, drain=False):
            # softmax denominators, one block behind the attention loop.
            # r_rep holds R replicated across partitions (row layout, indexed
            # by m along the free axis); the division happens at the rep
            # accumulation where m is the PARTITION axis, so R must transpose
            # into column layout [128, MCL]. Steady state uses a tiny DRAM
            # round-trip (two chained DMAs, zero engine cost, latency hidden
            # by the one-block lag). The post-loop drain can't hide that ~4us
            # latency, so it instead assembles the columns with DVE 32x32
            # StreamTranspose blocks (rows are replicated, so blocks with
            # matching partition bases pick out rt[32a+i, mcl] = r[mcl*128+
            # 32a+i]); the DVE is idle in the drain.
            h, mbi, ots, r_rep = st
            if drain:
                rt = rip.tile([P, MCL, 32], F32, name=f"rt{h}_{mbi}", tag="rt", bufs=1)
                for a in range(4):
                    row = r_rep[32 * a:32 * (a + 1), :].rearrange("p (m x) -> p m x", x=P)
                    for mcl in range(MCL):
                        nc.vector.transpose(
                            out=rt[32 * a:32 * (a + 1), mcl, :],
                            in_=row[:, mcl, 32 * a:32 * (a + 1)],
                        )
                rcinv = rip.tile([P, MCL], F32, name=f"rci{h}_{mbi}", tag="rci", bufs=3)
                nc.vector.reciprocal(out=rcinv, in_=rt[:, :, 0])
                return (h, mbi, ots, rcinv)
            rdram = drp.tile([mb], F32, name=f"rd{h}_{mbi}", tag="rd")
            nc.sync.dma_start(out=rdram[:], in_=r_rep[0:1, :])
            rcol = rip.tile([P, MCL], F32, name=f"rc{h}_{mbi}", tag="rc", bufs=3)
            nc.sync.dma_start(out=rcol, in_=rdram[:].rearrange("(c p) -> p c", p=P))
            rcinv = rip.tile([P, MCL], F32, name=f"rci{h}_{mbi}", tag="rci", bufs=3)
            nc.vector.reciprocal(out=rcinv, in_=rcol)
            return (h, mbi, ots, rcinv)

        # head 0 loads: DMA issue order = first-use order. The critical-path
        # pieces (gt0 + kT) go first; bo/ub and the rep-init DVE copies are
        # deferred past the projection emission (first needed ~40us in).
        gt_cur = load_w(0, GT, "gt")
        ub_sb = constp.tile([P, DC, h_cnt], BF, name="ub_sb", tag="ub")
        nc.sync.dma_start(out=ub_sb, in_=ub[:].rearrange("(c p) h -> p c h", p=P))
        load_input(kT_sb, kT, n)
        u_cur = load_w(0, U, "u")
        load_input(vT_sb, vT, n)
        load_input(qT_sb, qT, m)
        kgT_cur, vu_cur = project_head(0, gt_cur, u_cur)
        colb_sb = project_colb()

        bo_sb = constp.tile([P, d], F32, name="bo_sb", tag="bo")
        bo_ap = bo[:]
        nc.sync.dma_start(
            out=bo_sb,
            in_=bass.AP(tensor=bo_ap.tensor, offset=bo_ap.offset, ap=[[0, P], *bo_ap.ap]),
        )
        # rep accumulator, initialized with the (effective) output bias
        rep_sb = accp.tile([P, m // P, d], F32, name="rep_sb", tag="rep")
        for mc in range(m // P):
            nc.vector.tensor_copy(out=rep_sb[:, mc, :], in_=bo_sb)
        gt_next = u_next = kgT_next = vu_next = None
        pend_norm = None  # attention output awaiting softmax normalize
        pend_fp = None    # normalized output awaiting rep accumulation

        def emit_pv(h, mbi, rp_ps, et, ncc):
            for mcl in range(MCL):
                nc.tensor.matmul(
                    rp_ps[mcl],
                    lhsT=et[:, mcl * P:(mcl + 1) * P],
                    rhs=vu_cur[:, ncc, :],
                    start=(ncc == 0),
                    stop=(ncc == NCH - 1),
                )

        for h in range(h_cnt):
            for mbi in range(NMB):
                # ---- attention inner loop over n chunks ----
                # scores are emitted 2 iterations ahead of PV: the PV matmul's
                # stationary operand IS the exp output, so the TensorEngine
                # needs >= 2 score groups of runway for the ScalarE exp to
                # complete without exposing its latency
                rp_ps = [
                    pso.tile([P, d], F32, name=f"rp{h}_{mbi}_{mcl}", tag=f"rp{mcl}")
                    for mcl in range(MCL)
                ]
                esum = esp.tile([P, mb], F32, name=f"es{h}_{mbi}", tag="esum", bufs=3)
                ets = []
                for ncc in range(NCH):
                    st_ps = pst.tile([P, mb], F32, name=f"st{h}_{mbi}_{ncc}", tag="st")
                    for dc in range(DC):
                        nc.tensor.matmul(
                            st_ps,
                            lhsT=kgT_cur[:, dc, ncc * P:(ncc + 1) * P],
                            rhs=qT_sb[:, dc, mbi * mb:(mbi + 1) * mb],
                            start=(dc == 0),
                            stop=(dc == DC - 1),
                        )
                    et = etp.tile([P, mb], BF, name=f"et{h}_{mbi}_{ncc}", tag="et", bufs=6)
                    nc.scalar.activation(out=et, in_=st_ps, func=EXP,
                                         scale=inv_sqrt_d,
                                         bias=colb_sb[:, ncc, h:h + 1])
                    if ncc == 0:
                        nc.vector.tensor_copy(out=esum, in_=et)
                    else:
                        nc.vector.tensor_add(out=esum, in0=esum, in1=et)
                    ets.append(et)
                    if ncc >= 2:
                        emit_pv(h, mbi, rp_ps, ets[ncc - 2], ncc - 2)
                    # lag-1 softmax chain, emitted MID-loop so its DVE ops
                    # interleave with this block's esum adds instead of
                    # serializing after the last block (short pipeline drain).
                    # By ncc==5 the previous block's gpsimd row-sum reduce and
                    # the R-transpose DMA round-trip have long completed.
                    if ncc == 5 and pend_norm is not None:
                        pend_fp = normalize(pend_norm)
                        pend_norm = None
                    if ncc == 12 and pend_fp is not None:
                        final_acc(*pend_fp)
                        pend_fp = None
                emit_pv(h, mbi, rp_ps, ets[NCH - 2], NCH - 2)
                emit_pv(h, mbi, rp_ps, ets[NCH - 1], NCH - 1)

                # evacuate rep' psum to SBUF (unnormalized, f32) immediately:
                # frees the psum banks for the next block's PV groups so the
                # softmax chain can lag without holding the TensorEngine
                ots = otp.tile([P, MCL, d], F32, name=f"ots{h}_{mbi}", tag="ots")
                for mcl in range(MCL):
                    nc.vector.tensor_copy(out=ots[:, mcl, :], in_=rp_ps[mcl])

                # row sums on GpSimd (own FIFO, runs during the next block)
                r_rep = rip.tile([P, mb], F32, name=f"rr{h}_{mbi}", tag="rr", bufs=3)
                nc.gpsimd.partition_all_reduce(r_rep, esum[:], P, bass_isa.ReduceOp.add)

                # ---- lookahead emission: independent PE work ----
                if mbi == 0 and h + 1 < h_cnt:
                    gt_next = load_w(h + 1, GT, "gt")
                    u_next = load_w(h + 1, U, "u")
                if mbi == NMB - 1 and h + 1 < h_cnt:
                    kgT_next, vu_next = project_head(h + 1, gt_next, u_next)

                pend_norm = (h, mbi, ots, r_rep)

                if mbi == NMB - 1 and h + 1 < h_cnt:
                    kgT_cur, vu_cur = kgT_next, vu_next

        # drain: only the last block's softmax chain remains
        if pend_fp is not None:
            final_acc(*pend_fp, drain=True)
        if pend_norm is not None:
            final_acc(*normalize(pend_norm, drain=True), drain=True)

    if not nc.is_finalized():
        nc.finalize()
    return nc


def prepare_in_maps(k, v, q, Wk, bk, Wv, bv, Wq, bq, Wo, bo):
    """Shard + fuse + lay out the full inputs for the 8 cores (host numpy)."""
    bf16 = ml_dtypes.bfloat16
    f32 = np.float32
    f64 = np.float64
    h_cnt, d = Wk.shape[0], Wk.shape[1]
    # Wo rows are ordered d*H + h (d-major flatten): per-head slice h::H
    Wo_h = np.stack([Wo[h::h_cnt, :] for h in range(h_cnt)])  # [H, D, D]
    # fused score matrix and fused v/output projection (exact, f64)
    GT = np.stack([Wk[h].astype(f64) @ Wq[h].astype(f64).T for h in range(h_cnt)])
    U = np.stack([Wv[h].astype(f64) @ Wo_h[h].astype(f64) for h in range(h_cnt)])
    # score column-bias generators (bq term; bk and the row-constant terms
    # are invariant under softmax and dropped)
    ub = np.stack(
        [Wk[h].astype(f64) @ bq[h].astype(f64) for h in range(h_cnt)], axis=1
    ) * float(d) ** -0.5  # [D, H]
    # softmax rows sum to 1, so each head's bv contributes the constant
    # vector bv[h] @ U... fold all of it into the output bias
    bo_eff = bo.astype(f64) + sum(
        bv[h].astype(f64) @ Wo_h[h].astype(f64) for h in range(h_cnt)
    )
    shared = {
        "GT": np.ascontiguousarray(GT).astype(bf16),
        "U": np.ascontiguousarray(U).astype(bf16),
        "ub": np.ascontiguousarray(ub).astype(bf16),
        "bo": np.ascontiguousarray(bo_eff).astype(f32),
    }
    in_maps = []
    for b in range(k.shape[0]):
        in_maps.append({
            "kT": np.ascontiguousarray(k[b].T).astype(bf16),
            "vT": np.ascontiguousarray(v[b].T).astype(bf16),
            "qT": np.ascontiguousarray(q[b].T).astype(bf16),
            **shared,
        })
    return in_maps


def run(in_maps, trace=False):
    from concourse.bass_utils import run_bass_kernel_spmd

    nc = build_program()
    res = run_bass_kernel_spmd(nc, in_maps, core_ids=list(range(len(in_maps))), trace=trace)
    out = np.stack([np.asarray(r["out"], dtype=np.float32) for r in res.results])
    return out, res


def kernel(k, v, q, Wk, bk, Wv, bv, Wq, bq, Wo, bo):
    args = [np.asarray(a) for a in (k, v, q, Wk, bk, Wv, bv, Wq, bq, Wo, bo)]
    in_maps = prepare_in_maps(*args)
    out, _ = run(in_maps, trace=False)
    return out


# revision 25
# speedup vs baseline: 1.0088x; 1.0004x over previous
"""Trainium2 Bass kernel for nn_Attention (dense multi-head cross-attention).

Problem: B=8 batches, N=M=2048 seq, D=512 hidden, H=8 heads.
  per head h: k_h = k @ Wk[h] + bk[h]; v_h, q_h likewise
              out_h = softmax(q_h k_h^T / sqrt(D)) v_h
  rep = concat_d-major(out_h) @ Wo + bo

Sharding: data-parallel over batch -> one batch element per NeuronCore,
zero collectives. All 8 cores run the same program (SPMD) on their own
batch slice.

Bilinear fusion (host-side, exact in f64): the q/k/v/output projections
collapse into two D x D matrices per head, cutting device FLOPs ~17%:
  scores   = (q Wq + bq)(k Wk + bk)^T
           = q (Wq Wk^T) k^T + [row-const terms] + bq (k Wk)^T
    row-constant terms are invariant under softmax over n -> dropped;
    GT_h = Wk_h Wq_h^T is the fused score matrix, and the column bias
    colb[n] = k_n . (Wk_h bq_h)/sqrt(D) folds into the exp activation's
    per-partition bias. bk drops out of the computation entirely.
  out Wo_h = A (v Wv + bv) Wo_h = A v (Wv Wo_h) + (softmax rows sum to 1)
    U_h = Wv_h Wo_h fuses the v and output projections; bv contributes
    the constant bv_h Wo_h, folded into the output bias on the host.

Per-core algorithm (transposed layouts, no on-chip transposes):
  kgT[g,n] = GT^T kT        lhsT=GT[d,g] rhs=kT[d,n]
  vu [n,e] = v @ U          lhsT=vT[d,n] rhs=U[d,e]
  colb[n-part, h] per-head  lhsT=kT[d,n-chunk] rhs=ub[d,h]  (column layout)
  ST [n,m] = kgT^T qT       lhsT=kgT[g,n] rhs=qT[g,m]   (q used raw!)
  ET = exp(ST/sqrt(D) + colb)  (no max subtraction: scores ~ N(0,1))
  R  [m]   = sum_n ET       (DVE chunk-accumulate + GpSimd partition_all_reduce)
  rep'[m,e] = ET^T vu       lhsT=ET[n,m-chunk] rhs=vu[n,e]  (direct [m,d]!)
  rep[m,e] += rep'/R        (softmax normalization deferred past the PV matmul;
                             R transposed to column layout via a tiny DRAM
                             round-trip, 1/R is a per-partition scalar)

Scheduling (TensorE measured ~99% busy inside its span):
  - score groups are emitted 2 n-chunk iterations ahead of the PV groups:
    the PV stationary operand IS the exp output, so without runway the
    exp latency would stall the PE every iteration;
  - the 1/R scaling runs on the DVE (tensor_scalar_mul), never ScalarE,
    so it can't delay the next block's first exp;
  - the lag-1 normalize/accumulate chain is emitted MID-inner-loop so its
    DVE ops interleave with the esum adds, and the post-loop drain only
    handles the final block (with DVE StreamTranspose replacing the DRAM
    round-trip since nothing hides its latency there);
  - psum evacuation copies are emitted right after the tail PV groups so
    the accumulator banks free before the next block's start=True writes;
  - the next head's kgT/vu projections are emitted at the end of each
    head's last block (independent PE work covering the softmax tail).

All matmul operands are bf16 (f32 PSUM accumulation): measured end-to-end
rel err vs the f32 reference is ~4e-3. fp8 (DoubleRow) was evaluated and
rejected: e4m3's 3-bit mantissa adds >=3.5e-2 rel err, over the 2e-2 gate.
"""

import numpy as np
import ml_dtypes

P = 128
B, N, M, D, H = 8, 2048, 2048, 512, 8


def build_program(n=N, m=M, d=D, h_cnt=H, mb=512):
    import concourse.bass as bass
    import concourse.tile as tile
    from concourse import bacc, bass_isa, mybir

    BF = mybir.dt.bfloat16
    F32 = mybir.dt.float32
    EXP = mybir.ActivationFunctionType.Exp

    DC = d // P        # contraction chunk count
    NCH = n // P       # n chunks
    NMB = m // mb      # m blocks
    MCL = mb // P      # m chunks per m block
    NF = n // mb       # n free-blocks for kgT projection
    inv_sqrt_d = float(d) ** -0.5

    nc = bacc.Bacc()
    kT = nc.declare_dram_parameter("kT", [d, n], BF, isOutput=False)
    vT = nc.declare_dram_parameter("vT", [d, n], BF, isOutput=False)
    qT = nc.declare_dram_parameter("qT", [d, m], BF, isOutput=False)
    # GT = Wk_h Wq_h^T (fused score matrix), U = Wv_h Wo_h (fused v/out proj)
    GT = nc.declare_dram_parameter("GT", [h_cnt, d, d], BF, isOutput=False)
    U = nc.declare_dram_parameter("U", [h_cnt, d, d], BF, isOutput=False)
    # ub[:, h] = (Wk_h bq_h) / sqrt(D): score column-bias generators
    ub = nc.declare_dram_parameter("ub", [d, h_cnt], BF, isOutput=False)
    # bo here is bo + sum_h bv[h] @ Wo_h (bv folded on host)
    bo = nc.declare_dram_parameter("bo", [d], F32, isOutput=False)
    out = nc.declare_dram_parameter("out", [m, d], F32, isOutput=True)

    with (
        tile.TileContext(nc) as tc,
        tc.tile_pool(name="constp", bufs=1) as constp,
        tc.tile_pool(name="inp", bufs=1) as inp,
        tc.tile_pool(name="wts", bufs=2) as wts,
        tc.tile_pool(name="proj", bufs=1) as proj,
        tc.tile_pool(name="etp", bufs=6) as etp,
        tc.tile_pool(name="esp", bufs=3) as esp,
        tc.tile_pool(name="otp", bufs=3) as otp,
        tc.tile_pool(name="rip", bufs=2) as rip,
        tc.tile_pool(name="ftp", bufs=3) as ftp,
        tc.tile_pool(name="accp", bufs=1) as accp,
        tc.tile_pool(name="drp", bufs=3, space="DRAM") as drp,
        tc.tile_pool(name="pst", bufs=3, space="PSUM") as pst,
        tc.tile_pool(name="pso", bufs=1, space="PSUM") as pso,
        tc.tile_pool(name="pcb", bufs=1, space="PSUM") as pcb,
    ):
        # resident transposed inputs [d-chunk partitions, chunk, seq]
        kT_sb = inp.tile([P, DC, n], BF, name="kT_sb", tag="kT")
        vT_sb = inp.tile([P, DC, n], BF, name="vT_sb", tag="vT")
        qT_sb = inp.tile([P, DC, m], BF, name="qT_sb", tag="qT")

        def load_w(h, w_dram, tag, split=1):
            w_sb = wts.tile([P, DC, d], BF, name=f"{tag}{h}", tag=tag)
            src = w_dram[h].rearrange("(c p) e -> c p e", p=P)
            step = d // split
            for dc in range(DC):
                for s in range(split):
                    nc.sync.dma_start(
                        out=w_sb[:, dc, s * step:(s + 1) * step],
                        in_=src[dc][:, s * step:(s + 1) * step],
                    )
            return w_sb

        def load_input(x_sb, x_dram, length, split0=1):
            # ~128KB pieces spread across DMA queues so the first consumers
            # aren't gated on one 512KB-per-queue transfer. split0 halves the
            # first n-block's pieces further: with all 16 queues draining in
            # parallel, smaller first pieces complete sooner, letting head 0's
            # kg projection start earlier.
            src = x_dram[:].rearrange("(c p) n -> c p n", p=P)
            for nf in range(length // mb):
                step = mb // (split0 if nf == 0 else 1)
                for dc in range(DC):
                    for s in range(mb // step):
                        lo = nf * mb + s * step
                        nc.sync.dma_start(
                            out=x_sb[:, dc, lo:lo + step],
                            in_=src[dc][:, lo:lo + step],
                        )

        def project_colb():
            # per-head score column-bias, computed directly in column layout:
            # colb[p, ncc, h] = sum_d kT[d, ncc*P+p] * ub[d, h]
            colb_sb = constp.tile([P, NCH, h_cnt], F32, name="colb", tag="colb")
            for ncc in range(NCH):
                cps = pcb.tile([P, h_cnt], F32, name=f"cb{ncc}", tag="cb")
                for dc in range(DC):
                    nc.tensor.matmul(
                        cps,
                        lhsT=kT_sb[:, dc, ncc * P:(ncc + 1) * P],
                        rhs=ub_sb[:, dc, :],
                        start=(dc == 0),
                        stop=(dc == DC - 1),
                    )
                nc.scalar.copy(out=colb_sb[:, ncc, :], in_=cps)
            return colb_sb

        def project_head(h, gt_sb, u_sb):
            # kgT[g, n], bf16 (no bias: colb rides the exp activation).
            # nf-major so head 0's projection can start as soon as the first
            # kT DMA pieces land instead of waiting for the full tensor.
            kgT_sb = proj.tile([P, DC, n], BF, name=f"kgT{h}", tag="kgT")
            for nf in range(NF):
                for ec in range(DC):
                    ps = pst.tile([P, mb], F32, name=f"psk{h}_{ec}_{nf}", tag="st")
                    for dc in range(DC):
                        nc.tensor.matmul(
                            ps,
                            lhsT=gt_sb[:, dc, ec * P:(ec + 1) * P],
                            rhs=kT_sb[:, dc, nf * mb:(nf + 1) * mb],
                            start=(dc == 0),
                            stop=(dc == DC - 1),
                        )
                    nc.scalar.copy(out=kgT_sb[:, ec, nf * mb:(nf + 1) * mb], in_=ps)
            # vu[n, e], bf16 (no bias: bv folded into bo on the host)
            vu_sb = proj.tile([P, NCH, d], BF, name=f"vu{h}", tag="vu")
            for ncc in range(NCH):
                ps = pst.tile([P, d], F32, name=f"psv{h}_{ncc}", tag="st")
                for dc in range(DC):
                    nc.tensor.matmul(
                        ps,
                        lhsT=vT_sb[:, dc, ncc * P:(ncc + 1) * P],
                        rhs=u_sb[:, dc, :],
                        start=(dc == 0),
                        stop=(dc == DC - 1),
                    )
                nc.scalar.copy(out=vu_sb[:, ncc, :], in_=ps)
            return kgT_sb, vu_sb

        def final_acc(h, mbi, ots, rcinv, drain=False):
            # rep accumulation of the UNNORMALIZED attention output: m is the
            # partition axis and 1/R is a per-partition scalar. Both ops run
            # on the DVE: a ScalarE mul here would sit in the ScalarE queue
            # between this block's exps and the next block's first exp, and
            # the next block's first PV matmul (lhsT=et) waits on that exp.
            # In the post-loop drain there are no exps left, so the muls go
            # to the (idle) ScalarE to overlap with the DVE adds.
            for mcl in range(MCL):
                mc = mbi * MCL + mcl
                tmp = ftp.tile([P, d], F32, name=f"ft{h}_{mbi}_{mcl}", tag="ft")
                if drain:
                    nc.scalar.mul(out=tmp, in_=ots[:, mcl, :], mul=rcinv[:, mcl:mcl + 1])
                else:
                    nc.vector.tensor_scalar_mul(tmp, ots[:, mcl, :], rcinv[:, mcl:mcl + 1])
                nc.vector.tensor_add(out=rep_sb[:, mc, :], in0=rep_sb[:, mc, :], in1=tmp)
                if h == h_cnt - 1:
                    nc.sync.dma_start(
                        out=out[:].rearrange("(c p) e -> c p e", p=P)[mc],
                        in_=rep_sb[:, mc, :],
                    )

        def normalize(st, drain=False):
            # softmax denominators, one block behind the attention loop.
            # r_rep holds R replicated across partitions (row layout, indexed
            # by m along the free axis); the division happens at the rep
            # accumulation where m is the PARTITION axis, so R must transpose
            # into column layout [128, MCL]. Steady state uses a tiny DRAM
            # round-trip (two chained DMAs, zero engine cost, latency hidden
            # by the one-block lag). The post-loop drain can't hide that ~4us
            # latency, so it instead assembles the columns with DVE 32x32
            # StreamTranspose blocks (rows are replicated, so blocks with
            # matching partition bases pick out rt[32a+i, mcl] = r[mcl*128+
            # 32a+i]); the DVE is idle in the drain.
            h, mbi, ots, r_rep = st
            if drain:
                rt = rip.tile([P, MCL, 32], F32, name=f"rt{h}_{mbi}", tag="rt", bufs=1)
                for a in range(4):
                    row = r_rep[32 * a:32 * (a + 1), :].rearrange("p (m x) -> p m x", x=P)
                    for mcl in range(MCL):
                        nc.vector.transpose(
                            out=rt[32 * a:32 * (a + 1), mcl, :],
                            in_=row[:, mcl, 32 * a:32 * (a + 1)],
                        )
                rcinv = rip.tile([P, MCL], F32, name=f"rci{h}_{mbi}", tag="rci", bufs=3)
                nc.vector.reciprocal(out=rcinv, in_=rt[:, :, 0])
                return (h, mbi, ots, rcinv)
            rdram = drp.tile([mb], F32, name=f"rd{h}_{mbi}", tag="rd")
            nc.sync.dma_start(out=rdram[:], in_=r_rep[0:1, :])
            rcol = rip.tile([P, MCL], F32, name=f"rc{h}_{mbi}", tag="rc", bufs=3)
            nc.sync.dma_start(out=rcol, in_=rdram[:].rearrange("(c p) -> p c", p=P))
            rcinv = rip.tile([P, MCL], F32, name=f"rci{h}_{mbi}", tag="rci", bufs=3)
            nc.vector.reciprocal(out=rcinv, in_=rcol)
            return (h, mbi, ots, rcinv)

        # head 0 loads: DMA issue order = first-use order. The critical-path
        # pieces (gt0 + kT) go first; bo/ub and the rep-init DVE copies are
        # deferred past the projection emission (first needed ~40us in).
        gt_cur = load_w(0, GT, "gt")
        ub_sb = constp.tile([P, DC, h_cnt], BF, name="ub_sb", tag="ub")
        nc.sync.dma_start(out=ub_sb, in_=ub[:].rearrange("(c p) h -> p c h", p=P))
        load_input(kT_sb, kT, n)
        u_cur = load_w(0, U, "u")
        load_input(vT_sb, vT, n)
        load_input(qT_sb, qT, m)
        kgT_cur, vu_cur = project_head(0, gt_cur, u_cur)
        colb_sb = project_colb()

        bo_sb = constp.tile([P, d], F32, name="bo_sb", tag="bo")
        bo_ap = bo[:]
        nc.sync.dma_start(
            out=bo_sb,
            in_=bass.AP(tensor=bo_ap.tensor, offset=bo_ap.offset, ap=[[0, P], *bo_ap.ap]),
        )
        # rep accumulator, initialized with the (effective) output bias
        rep_sb = accp.tile([P, m // P, d], F32, name="rep_sb", tag="rep")
        for mc in range(m // P):
            nc.vector.tensor_copy(out=rep_sb[:, mc, :], in_=bo_sb)
        gt_next = u_next = kgT_next = vu_next = None
        pend_norm = None  # attention output awaiting softmax normalize
        pend_fp = None    # normalized output awaiting rep accumulation

        def emit_pv(h, mbi, rp_ps, et, ncc):
            for mcl in range(MCL):
                nc.tensor.matmul(
                    rp_ps[mcl],
                    lhsT=et[:, mcl * P:(mcl + 1) * P],
                    rhs=vu_cur[:, ncc, :],
                    start=(ncc == 0),
                    stop=(ncc == NCH - 1),
                )

        for h in range(h_cnt):
            for mbi in range(NMB):
                # ---- attention inner loop over n chunks ----
                # scores are emitted PV_LAG iterations ahead of PV: the PV
                # matmul's stationary operand IS the exp output, so the
                # TensorEngine needs runway for the ScalarE exp to complete
                # without exposing its latency. A deep lag (6) also pulls the
                # last esum add several score-groups before block end, so the
                # GpSimd row-sum reduce mostly overlaps the PV tail -- which
                # is what bounds the post-loop drain of the final block.
                PV_LAG = 6
                rp_ps = [
                    pso.tile([P, d], F32, name=f"rp{h}_{mbi}_{mcl}", tag=f"rp{mcl}")
                    for mcl in range(MCL)
                ]
                esum = esp.tile([P, mb], F32, name=f"es{h}_{mbi}", tag="esum", bufs=3)
                ets = []
                for ncc in range(NCH):
                    st_ps = pst.tile([P, mb], F32, name=f"st{h}_{mbi}_{ncc}", tag="st")
                    for dc in range(DC):
                        nc.tensor.matmul(
                            st_ps,
                            lhsT=kgT_cur[:, dc, ncc * P:(ncc + 1) * P],
                            rhs=qT_sb[:, dc, mbi * mb:(mbi + 1) * mb],
                            start=(dc == 0),
                            stop=(dc == DC - 1),
                        )
                    et = etp.tile([P, mb], BF, name=f"et{h}_{mbi}_{ncc}", tag="et", bufs=8)
                    nc.scalar.activation(out=et, in_=st_ps, func=EXP,
                                         scale=inv_sqrt_d,
                                         bias=colb_sb[:, ncc, h:h + 1])
                    if ncc == 0:
                        nc.vector.tensor_copy(out=esum, in_=et)
                    else:
                        nc.vector.tensor_add(out=esum, in0=esum, in1=et)
                    ets.append(et)
                    if ncc >= PV_LAG:
                        emit_pv(h, mbi, rp_ps, ets[ncc - PV_LAG], ncc - PV_LAG)
                    # lag-1 softmax chain, emitted MID-loop so its DVE ops
                    # interleave with this block's esum adds instead of
                    # serializing after the last block (short pipeline drain).
                    # By ncc==5 the previous block's gpsimd row-sum reduce and
                    # the R-transpose DMA round-trip have long completed.
                    if ncc == 5 and pend_norm is not None:
                        pend_fp = normalize(pend_norm)
                        pend_norm = None
                    if ncc == 12 and pend_fp is not None:
                        final_acc(*pend_fp)
                        pend_fp = None
                for j in range(NCH - PV_LAG, NCH):
                    emit_pv(h, mbi, rp_ps, ets[j], j)

                # evacuate rep' psum to SBUF (unnormalized, f32) immediately:
                # frees the psum banks for the next block's PV groups so the
                # softmax chain can lag without holding the TensorEngine.
                # DVE normally (prompt, can't delay the next block's exps);
                # ScalarE for the final block so the drain's StreamTranspose/
                # reciprocal run on the DVE in parallel with the evacuation.
                last = (h == h_cnt - 1 and mbi == NMB - 1)
                ots = otp.tile([P, MCL, d], F32, name=f"ots{h}_{mbi}", tag="ots")
                for mcl in range(MCL):
                    if last:
                        nc.scalar.copy(out=ots[:, mcl, :], in_=rp_ps[mcl])
                    else:
                        nc.vector.tensor_copy(out=ots[:, mcl, :], in_=rp_ps[mcl])

                # row sums on GpSimd (own FIFO, runs during the next block)
                r_rep = rip.tile([P, mb], F32, name=f"rr{h}_{mbi}", tag="rr", bufs=3)
                nc.gpsimd.partition_all_reduce(r_rep, esum[:], P, bass_isa.ReduceOp.add)

                # ---- lookahead emission: independent PE work ----
                if mbi == 0 and h + 1 < h_cnt:
                    gt_next = load_w(h + 1, GT, "gt")
                    u_next = load_w(h + 1, U, "u")
                if mbi == NMB - 1 and h + 1 < h_cnt:
                    kgT_next, vu_next = project_head(h + 1, gt_next, u_next)

                pend_norm = (h, mbi, ots, r_rep)

                if mbi == NMB - 1 and h + 1 < h_cnt:
                    kgT_cur, vu_cur = kgT_next, vu_next

        # drain: only the last block's softmax chain remains
        if pend_fp is not None:
            final_acc(*pend_fp, drain=True)
        if pend_norm is not None:
            final_acc(*normalize(pend_norm, drain=True), drain=True)

    if not nc.is_finalized():
        nc.finalize()
    return nc


def prepare_in_maps(k, v, q, Wk, bk, Wv, bv, Wq, bq, Wo, bo):
    """Shard + fuse + lay out the full inputs for the 8 cores (host numpy)."""
    bf16 = ml_dtypes.bfloat16
    f32 = np.float32
    f64 = np.float64
    h_cnt, d = Wk.shape[0], Wk.shape[1]
    # Wo rows are ordered d*H + h (d-major flatten): per-head slice h::H
    Wo_h = np.stack([Wo[h::h_cnt, :] for h in range(h_cnt)])  # [H, D, D]
    # fused score matrix and fused v/output projection (exact, f64)
    GT = np.stack([Wk[h].astype(f64) @ Wq[h].astype(f64).T for h in range(h_cnt)])
    U = np.stack([Wv[h].astype(f64) @ Wo_h[h].astype(f64) for h in range(h_cnt)])
    # score column-bias generators (bq term; bk and the row-constant terms
    # are invariant under softmax and dropped)
    ub = np.stack(
        [Wk[h].astype(f64) @ bq[h].astype(f64) for h in range(h_cnt)], axis=1
    ) * float(d) ** -0.5  # [D, H]
    # softmax rows sum to 1, so each head's bv contributes the constant
    # vector bv[h] @ U... fold all of it into the output bias
    bo_eff = bo.astype(f64) + sum(
        bv[h].astype(f64) @ Wo_h[h].astype(f64) for h in range(h_cnt)
    )
    shared = {
        "GT": np.ascontiguousarray(GT).astype(bf16),
        "U": np.ascontiguousarray(U).astype(bf16),
        "ub": np.ascontiguousarray(ub).astype(bf16),
        "bo": np.ascontiguousarray(bo_eff).astype(f32),
    }
    in_maps = []
    for b in range(k.shape[0]):
        in_maps.append({
            "kT": np.ascontiguousarray(k[b].T).astype(bf16),
            "vT": np.ascontiguousarray(v[b].T).astype(bf16),
            "qT": np.ascontiguousarray(q[b].T).astype(bf16),
            **shared,
        })
    return in_maps


def run(in_maps, trace=False):
    from concourse.bass_utils import run_bass_kernel_spmd

    nc = build_program()
    res = run_bass_kernel_spmd(nc, in_maps, core_ids=list(range(len(in_maps))), trace=trace)
    out = np.stack([np.asarray(r["out"], dtype=np.float32) for r in res.results])
    return out, res


def kernel(k, v, q, Wk, bk, Wv, bv, Wq, bq, Wo, bo):
    args = [np.asarray(a) for a in (k, v, q, Wk, bk, Wv, bv, Wq, bq, Wo, bo)]
    in_maps = prepare_in_maps(*args)
    out, _ = run(in_maps, trace=False)
    return out


# revision 28
# speedup vs baseline: 1.0117x; 1.0030x over previous
"""Trainium2 Bass kernel for nn_Attention (dense multi-head cross-attention).

Problem: B=8 batches, N=M=2048 seq, D=512 hidden, H=8 heads.
  per head h: k_h = k @ Wk[h] + bk[h]; v_h, q_h likewise
              out_h = softmax(q_h k_h^T / sqrt(D)) v_h
  rep = concat_d-major(out_h) @ Wo + bo

Sharding: data-parallel over batch -> one batch element per NeuronCore,
zero collectives. All 8 cores run the same program (SPMD) on their own
batch slice.

Bilinear fusion (host-side, exact in f64): the q/k/v/output projections
collapse into two D x D matrices per head, cutting device FLOPs ~17%:
  scores   = (q Wq + bq)(k Wk + bk)^T
           = q (Wq Wk^T) k^T + [row-const terms] + bq (k Wk)^T
    row-constant terms are invariant under softmax over n -> dropped;
    GT_h = Wk_h Wq_h^T is the fused score matrix, and the column bias
    colb[n] = k_n . (Wk_h bq_h)/sqrt(D) folds into the exp activation's
    per-partition bias. bk drops out of the computation entirely.
  out Wo_h = A (v Wv + bv) Wo_h = A v (Wv Wo_h) + (softmax rows sum to 1)
    U_h = Wv_h Wo_h fuses the v and output projections; bv contributes
    the constant bv_h Wo_h, folded into the output bias on the host.

Per-core algorithm (transposed layouts, no on-chip transposes):
  kgT[g,n] = GT^T kT        lhsT=GT[d,g] rhs=kT[d,n]
  vu [n,e] = v @ U          lhsT=vT[d,n] rhs=U[d,e]
  colb[n-part, h] per-head  lhsT=kT[d,n-chunk] rhs=ub[d,h]  (column layout)
  ST [n,m] = kgT^T qT       lhsT=kgT[g,n] rhs=qT[g,m]   (q used raw!)
  ET = exp(ST/sqrt(D) + colb)  (no max subtraction: scores ~ N(0,1))
  R  [m]   = sum_n ET       (DVE chunk-accumulate + GpSimd partition_all_reduce)
  rep'[m,e] = ET^T vu       lhsT=ET[n,m-chunk] rhs=vu[n,e]  (direct [m,d]!)
  rep[m,e] += rep'/R        (softmax normalization deferred past the PV matmul;
                             R transposed to column layout via a tiny DRAM
                             round-trip, 1/R is a per-partition scalar)

Scheduling (TensorE measured ~99% busy inside its span):
  - score groups are emitted 2 n-chunk iterations ahead of the PV groups:
    the PV stationary operand IS the exp output, so without runway the
    exp latency would stall the PE every iteration;
  - the 1/R scaling runs on the DVE (tensor_scalar_mul), never ScalarE,
    so it can't delay the next block's first exp;
  - the lag-1 normalize/accumulate chain is emitted MID-inner-loop so its
    DVE ops interleave with the esum adds, and the post-loop drain only
    handles the final block (with DVE StreamTranspose replacing the DRAM
    round-trip since nothing hides its latency there);
  - psum evacuation copies are emitted right after the tail PV groups so
    the accumulator banks free before the next block's start=True writes;
  - the next head's kgT/vu projections are emitted at the end of each
    head's last block (independent PE work covering the softmax tail).

All matmul operands are bf16 (f32 PSUM accumulation): measured end-to-end
rel err vs the f32 reference is ~4e-3. fp8 (DoubleRow) was evaluated and
rejected: e4m3's 3-bit mantissa adds >=3.5e-2 rel err, over the 2e-2 gate.
"""

import numpy as np
import ml_dtypes

P = 128
B, N, M, D, H = 8, 2048, 2048, 512, 8


def build_program(n=N, m=M, d=D, h_cnt=H, mb=512):
    import concourse.bass as bass
    import concourse.tile as tile
    from concourse import bacc, bass_isa, mybir

    BF = mybir.dt.bfloat16
    F32 = mybir.dt.float32
    EXP = mybir.ActivationFunctionType.Exp

    DC = d // P        # contraction chunk count
    NCH = n // P       # n chunks
    NMB = m // mb      # m blocks
    MCL = mb // P      # m chunks per m block
    NF = n // mb       # n free-blocks for kgT projection
    inv_sqrt_d = float(d) ** -0.5

    nc = bacc.Bacc()
    kT = nc.declare_dram_parameter("kT", [d, n], BF, isOutput=False)
    vT = nc.declare_dram_parameter("vT", [d, n], BF, isOutput=False)
    qT = nc.declare_dram_parameter("qT", [d, m], BF, isOutput=False)
    # GT = Wk_h Wq_h^T (fused score matrix), U = Wv_h Wo_h (fused v/out proj)
    GT = nc.declare_dram_parameter("GT", [h_cnt, d, d], BF, isOutput=False)
    U = nc.declare_dram_parameter("U", [h_cnt, d, d], BF, isOutput=False)
    # ub[:, h] = (Wk_h bq_h) / sqrt(D): score column-bias generators
    ub = nc.declare_dram_parameter("ub", [d, h_cnt], BF, isOutput=False)
    # bo here is bo + sum_h bv[h] @ Wo_h (bv folded on host)
    bo = nc.declare_dram_parameter("bo", [d], F32, isOutput=False)
    out = nc.declare_dram_parameter("out", [m, d], F32, isOutput=True)

    with (
        tile.TileContext(nc) as tc,
        tc.tile_pool(name="constp", bufs=1) as constp,
        tc.tile_pool(name="inp", bufs=1) as inp,
        tc.tile_pool(name="wts", bufs=2) as wts,
        tc.tile_pool(name="proj", bufs=1) as proj,
        tc.tile_pool(name="etp", bufs=6) as etp,
        tc.tile_pool(name="esp", bufs=3) as esp,
        tc.tile_pool(name="otp", bufs=3) as otp,
        tc.tile_pool(name="rip", bufs=2) as rip,
        tc.tile_pool(name="ftp", bufs=3) as ftp,
        tc.tile_pool(name="accp", bufs=1) as accp,
        tc.tile_pool(name="drp", bufs=3, space="DRAM") as drp,
        tc.tile_pool(name="pst", bufs=3, space="PSUM") as pst,
        tc.tile_pool(name="pso", bufs=1, space="PSUM") as pso,
        tc.tile_pool(name="pcb", bufs=1, space="PSUM") as pcb,
    ):
        # resident transposed inputs [d-chunk partitions, chunk, seq]
        kT_sb = inp.tile([P, DC, n], BF, name="kT_sb", tag="kT")
        vT_sb = inp.tile([P, DC, n], BF, name="vT_sb", tag="vT")
        qT_sb = inp.tile([P, DC, m], BF, name="qT_sb", tag="qT")

        def load_w(h, w_dram, tag, split=1):
            w_sb = wts.tile([P, DC, d], BF, name=f"{tag}{h}", tag=tag)
            src = w_dram[h].rearrange("(c p) e -> c p e", p=P)
            step = d // split
            for dc in range(DC):
                for s in range(split):
                    nc.sync.dma_start(
                        out=w_sb[:, dc, s * step:(s + 1) * step],
                        in_=src[dc][:, s * step:(s + 1) * step],
                    )
            return w_sb

        def load_input(x_sb, x_dram, length, split0=1):
            # ~128KB pieces spread across DMA queues so the first consumers
            # aren't gated on one 512KB-per-queue transfer. split0 halves the
            # first n-block's pieces further: with all 16 queues draining in
            # parallel, smaller first pieces complete sooner, letting head 0's
            # kg projection start earlier.
            src = x_dram[:].rearrange("(c p) n -> c p n", p=P)
            for nf in range(length // mb):
                step = mb // (split0 if nf == 0 else 1)
                for dc in range(DC):
                    for s in range(mb // step):
                        lo = nf * mb + s * step
                        nc.sync.dma_start(
                            out=x_sb[:, dc, lo:lo + step],
                            in_=src[dc][:, lo:lo + step],
                        )

        def project_colb():
            # per-head score column-bias, computed directly in column layout:
            # colb[p, ncc, h] = sum_d kT[d, ncc*P+p] * ub[d, h]
            colb_sb = constp.tile([P, NCH, h_cnt], F32, name="colb", tag="colb")
            for ncc in range(NCH):
                cps = pcb.tile([P, h_cnt], F32, name=f"cb{ncc}", tag="cb")
                for dc in range(DC):
                    nc.tensor.matmul(
                        cps,
                        lhsT=kT_sb[:, dc, ncc * P:(ncc + 1) * P],
                        rhs=ub_sb[:, dc, :],
                        start=(dc == 0),
                        stop=(dc == DC - 1),
                    )
                nc.scalar.copy(out=colb_sb[:, ncc, :], in_=cps)
            return colb_sb

        def project_head(h, gt_sb, u_sb):
            # kgT[g, n], bf16 (no bias: colb rides the exp activation).
            # nf-major so head 0's projection can start as soon as the first
            # kT DMA pieces land instead of waiting for the full tensor.
            kgT_sb = proj.tile([P, DC, n], BF, name=f"kgT{h}", tag="kgT")
            for nf in range(NF):
                for ec in range(DC):
                    ps = pst.tile([P, mb], F32, name=f"psk{h}_{ec}_{nf}", tag="st")
                    for dc in range(DC):
                        nc.tensor.matmul(
                            ps,
                            lhsT=gt_sb[:, dc, ec * P:(ec + 1) * P],
                            rhs=kT_sb[:, dc, nf * mb:(nf + 1) * mb],
                            start=(dc == 0),
                            stop=(dc == DC - 1),
                        )
                    nc.scalar.copy(out=kgT_sb[:, ec, nf * mb:(nf + 1) * mb], in_=ps)
            # vu[n, e], bf16 (no bias: bv folded into bo on the host)
            vu_sb = proj.tile([P, NCH, d], BF, name=f"vu{h}", tag="vu")
            for ncc in range(NCH):
                ps = pst.tile([P, d], F32, name=f"psv{h}_{ncc}", tag="st")
                for dc in range(DC):
                    nc.tensor.matmul(
                        ps,
                        lhsT=vT_sb[:, dc, ncc * P:(ncc + 1) * P],
                        rhs=u_sb[:, dc, :],
                        start=(dc == 0),
                        stop=(dc == DC - 1),
                    )
                nc.scalar.copy(out=vu_sb[:, ncc, :], in_=ps)
            return kgT_sb, vu_sb

        def final_acc(h, mbi, ots, rcinv, drain=False):
            # rep accumulation of the UNNORMALIZED attention output: m is the
            # partition axis and 1/R is a per-partition scalar. Both ops run
            # on the DVE: a ScalarE mul here would sit in the ScalarE queue
            # between this block's exps and the next block's first exp, and
            # the next block's first PV matmul (lhsT=et) waits on that exp.
            # In the post-loop drain there are no exps left, so the muls go
            # to the (idle) ScalarE to overlap with the DVE adds.
            for mcl in range(MCL):
                mc = mbi * MCL + mcl
                src = ots[mcl] if isinstance(ots, list) else ots[:, mcl, :]
                tmp = ftp.tile([P, d], F32, name=f"ft{h}_{mbi}_{mcl}", tag="ft")
                if drain:
                    nc.scalar.mul(out=tmp, in_=src, mul=rcinv[:, mcl:mcl + 1])
                else:
                    nc.vector.tensor_scalar_mul(tmp, src, rcinv[:, mcl:mcl + 1])
                nc.vector.tensor_add(out=rep_sb[:, mc, :], in0=rep_sb[:, mc, :], in1=tmp)
                if h == h_cnt - 1:
                    nc.sync.dma_start(
                        out=out[:].rearrange("(c p) e -> c p e", p=P)[mc],
                        in_=rep_sb[:, mc, :],
                    )

        def normalize(st, drain=False):
            # softmax denominators, one block behind the attention loop.
            # r_rep holds R replicated across partitions (row layout, indexed
            # by m along the free axis); the division happens at the rep
            # accumulation where m is the PARTITION axis, so R must transpose
            # into column layout [128, MCL]. Steady state uses a tiny DRAM
            # round-trip (two chained DMAs, zero engine cost, latency hidden
            # by the one-block lag). The post-loop drain can't hide that ~4us
            # latency, so it instead assembles the columns with DVE 32x32
            # StreamTranspose blocks (rows are replicated, so blocks with
            # matching partition bases pick out rt[32a+i, mcl] = r[mcl*128+
            # 32a+i]); the DVE is idle in the drain.
            h, mbi, ots, r_rep = st
            if drain:
                rt = rip.tile([P, MCL, 32], F32, name=f"rt{h}_{mbi}", tag="rt", bufs=1)
                for a in range(4):
                    row = r_rep[32 * a:32 * (a + 1), :].rearrange("p (m x) -> p m x", x=P)
                    for mcl in range(MCL):
                        nc.vector.transpose(
                            out=rt[32 * a:32 * (a + 1), mcl, :],
                            in_=row[:, mcl, 32 * a:32 * (a + 1)],
                        )
                rcinv = rip.tile([P, MCL], F32, name=f"rci{h}_{mbi}", tag="rci", bufs=3)
                nc.vector.reciprocal(out=rcinv, in_=rt[:, :, 0])
                return (h, mbi, ots, rcinv)
            rdram = drp.tile([mb], F32, name=f"rd{h}_{mbi}", tag="rd")
            nc.sync.dma_start(out=rdram[:], in_=r_rep[0:1, :])
            rcol = rip.tile([P, MCL], F32, name=f"rc{h}_{mbi}", tag="rc", bufs=3)
            nc.sync.dma_start(out=rcol, in_=rdram[:].rearrange("(c p) -> p c", p=P))
            rcinv = rip.tile([P, MCL], F32, name=f"rci{h}_{mbi}", tag="rci", bufs=3)
            nc.vector.reciprocal(out=rcinv, in_=rcol)
            return (h, mbi, ots, rcinv)

        # head 0 loads: DMA issue order = first-use order. The critical-path
        # pieces (gt0 + kT) go first; bo/ub and the rep-init DVE copies are
        # deferred past the projection emission (first needed ~40us in).
        gt_cur = load_w(0, GT, "gt")
        load_input(kT_sb, kT, n)
        ub_sb = constp.tile([P, DC, h_cnt], BF, name="ub_sb", tag="ub")
        nc.sync.dma_start(out=ub_sb, in_=ub[:].rearrange("(c p) h -> p c h", p=P))
        u_cur = load_w(0, U, "u")
        load_input(vT_sb, vT, n)
        load_input(qT_sb, qT, m)
        kgT_cur, vu_cur = project_head(0, gt_cur, u_cur)
        colb_sb = project_colb()

        bo_sb = constp.tile([P, d], F32, name="bo_sb", tag="bo")
        bo_ap = bo[:]
        nc.sync.dma_start(
            out=bo_sb,
            in_=bass.AP(tensor=bo_ap.tensor, offset=bo_ap.offset, ap=[[0, P], *bo_ap.ap]),
        )
        # rep accumulator, initialized with the (effective) output bias
        rep_sb = accp.tile([P, m // P, d], F32, name="rep_sb", tag="rep")
        for mc in range(m // P):
            nc.vector.tensor_copy(out=rep_sb[:, mc, :], in_=bo_sb)
        gt_next = u_next = kgT_next = vu_next = None
        pend_norm = None  # attention output awaiting softmax normalize
        pend_fp = None    # normalized output awaiting rep accumulation

        def emit_pv(h, mbi, rp_ps, et, ncc):
            for mcl in range(MCL):
                nc.tensor.matmul(
                    rp_ps[mcl],
                    lhsT=et[:, mcl * P:(mcl + 1) * P],
                    rhs=vu_cur[:, ncc, :],
                    start=(ncc == 0),
                    stop=(ncc == NCH - 1),
                )

        for h in range(h_cnt):
            for mbi in range(NMB):
                # ---- attention inner loop over n chunks ----
                # scores are emitted PV_LAG iterations ahead of PV: the PV
                # matmul's stationary operand IS the exp output, so the
                # TensorEngine needs runway for the ScalarE exp to complete
                # without exposing its latency. A deep lag (6) also pulls the
                # last esum add several score-groups before block end, so the
                # GpSimd row-sum reduce mostly overlaps the PV tail -- which
                # is what bounds the post-loop drain of the final block.
                PV_LAG = 6
                rp_ps = [
                    pso.tile([P, d], F32, name=f"rp{h}_{mbi}_{mcl}", tag=f"rp{mcl}")
                    for mcl in range(MCL)
                ]
                esum = esp.tile([P, mb], F32, name=f"es{h}_{mbi}", tag="esum", bufs=3)
                ets = []
                for ncc in range(NCH):
                    st_ps = pst.tile([P, mb], F32, name=f"st{h}_{mbi}_{ncc}", tag="st")
                    for dc in range(DC):
                        nc.tensor.matmul(
                            st_ps,
                            lhsT=kgT_cur[:, dc, ncc * P:(ncc + 1) * P],
                            rhs=qT_sb[:, dc, mbi * mb:(mbi + 1) * mb],
                            start=(dc == 0),
                            stop=(dc == DC - 1),
                        )
                    et = etp.tile([P, mb], BF, name=f"et{h}_{mbi}_{ncc}", tag="et", bufs=8)
                    nc.scalar.activation(out=et, in_=st_ps, func=EXP,
                                         scale=inv_sqrt_d,
                                         bias=colb_sb[:, ncc, h:h + 1])
                    if ncc == 0:
                        nc.vector.tensor_copy(out=esum, in_=et)
                    else:
                        nc.vector.tensor_add(out=esum, in0=esum, in1=et)
                    ets.append(et)
                    if ncc >= PV_LAG:
                        emit_pv(h, mbi, rp_ps, ets[ncc - PV_LAG], ncc - PV_LAG)
                    # lag-1 softmax chain, emitted MID-loop so its DVE ops
                    # interleave with this block's esum adds instead of
                    # serializing after the last block (short pipeline drain).
                    # By ncc==5 the previous block's gpsimd row-sum reduce and
                    # the R-transpose DMA round-trip have long completed.
                    if ncc == 5 and pend_norm is not None:
                        pend_fp = normalize(pend_norm)
                        pend_norm = None
                    if ncc == 12 and pend_fp is not None:
                        final_acc(*pend_fp)
                        pend_fp = None
                for j in range(NCH - PV_LAG, NCH):
                    emit_pv(h, mbi, rp_ps, ets[j], j)

                # evacuate rep' psum to SBUF (unnormalized, f32) immediately:
                # frees the psum banks for the next block's PV groups so the
                # softmax chain can lag without holding the TensorEngine.
                # The final block skips the evacuation: no block follows, so
                # the drain's 1/R muls read the PV psum banks directly.
                if h == h_cnt - 1 and mbi == NMB - 1:
                    ots = rp_ps
                else:
                    ots = otp.tile([P, MCL, d], F32, name=f"ots{h}_{mbi}", tag="ots")
                    for mcl in range(MCL):
                        nc.vector.tensor_copy(out=ots[:, mcl, :], in_=rp_ps[mcl])

                # row sums on GpSimd (own FIFO, runs during the next block)
                r_rep = rip.tile([P, mb], F32, name=f"rr{h}_{mbi}", tag="rr", bufs=3)
                nc.gpsimd.partition_all_reduce(r_rep, esum[:], P, bass_isa.ReduceOp.add)

                # ---- lookahead emission: independent PE work ----
                if mbi == 0 and h + 1 < h_cnt:
                    gt_next = load_w(h + 1, GT, "gt")
                    u_next = load_w(h + 1, U, "u")
                if mbi == NMB - 1 and h + 1 < h_cnt:
                    kgT_next, vu_next = project_head(h + 1, gt_next, u_next)

                pend_norm = (h, mbi, ots, r_rep)

                if mbi == NMB - 1 and h + 1 < h_cnt:
                    kgT_cur, vu_cur = kgT_next, vu_next

        # drain: only the last block's softmax chain remains
        if pend_fp is not None:
            final_acc(*pend_fp, drain=True)
        if pend_norm is not None:
            final_acc(*normalize(pend_norm, drain=True), drain=True)

    if not nc.is_finalized():
        nc.finalize()
    return nc


def prepare_in_maps(k, v, q, Wk, bk, Wv, bv, Wq, bq, Wo, bo):
    """Shard + fuse + lay out the full inputs for the 8 cores (host numpy)."""
    bf16 = ml_dtypes.bfloat16
    f32 = np.float32
    f64 = np.float64
    h_cnt, d = Wk.shape[0], Wk.shape[1]
    # Wo rows are ordered d*H + h (d-major flatten): per-head slice h::H
    Wo_h = np.stack([Wo[h::h_cnt, :] for h in range(h_cnt)])  # [H, D, D]
    # fused score matrix and fused v/output projection (exact, f64)
    GT = np.stack([Wk[h].astype(f64) @ Wq[h].astype(f64).T for h in range(h_cnt)])
    U = np.stack([Wv[h].astype(f64) @ Wo_h[h].astype(f64) for h in range(h_cnt)])
    # score column-bias generators (bq term; bk and the row-constant terms
    # are invariant under softmax and dropped)
    ub = np.stack(
        [Wk[h].astype(f64) @ bq[h].astype(f64) for h in range(h_cnt)], axis=1
    ) * float(d) ** -0.5  # [D, H]
    # softmax rows sum to 1, so each head's bv contributes the constant
    # vector bv[h] @ U... fold all of it into the output bias
    bo_eff = bo.astype(f64) + sum(
        bv[h].astype(f64) @ Wo_h[h].astype(f64) for h in range(h_cnt)
    )
    shared = {
        "GT": np.ascontiguousarray(GT).astype(bf16),
        "U": np.ascontiguousarray(U).astype(bf16),
        "ub": np.ascontiguousarray(ub).astype(bf16),
        "bo": np.ascontiguousarray(bo_eff).astype(f32),
    }
    in_maps = []
    for b in range(k.shape[0]):
        in_maps.append({
            "kT": np.ascontiguousarray(k[b].T).astype(bf16),
            "vT": np.ascontiguousarray(v[b].T).astype(bf16),
            "qT": np.ascontiguousarray(q[b].T).astype(bf16),
            **shared,
        })
    return in_maps


def run(in_maps, trace=False):
    from concourse.bass_utils import run_bass_kernel_spmd

    nc = build_program()
    res = run_bass_kernel_spmd(nc, in_maps, core_ids=list(range(len(in_maps))), trace=trace)
    out = np.stack([np.asarray(r["out"], dtype=np.float32) for r in res.results])
    return out, res


def kernel(k, v, q, Wk, bk, Wv, bv, Wq, bq, Wo, bo):
    args = [np.asarray(a) for a in (k, v, q, Wk, bk, Wv, bv, Wq, bq, Wo, bo)]
    in_maps = prepare_in_maps(*args)
    out, _ = run(in_maps, trace=False)
    return out
